# revision 26
# baseline (speedup 1.0000x reference)
"""DMN forward on 8 Trainium2 NeuronCores (Bass/Tile), v3 (bf16).

Sharding: batch rows 8/core for fact+question encoding and episodic memory;
decode GRU replicated on all cores, fc/log-softmax vocab-sharded 4000
columns/core. One tiny AllGather per decode step carries per-half
(max, argmax-token, sumexp); a dummy warmup AllGather early in the kernel
absorbs the first-collective latency while the fact GRU runs.

v3 structure:
- all matmuls bf16 (host-validated: relnorm ~1e-3 vs fp32 ref, gate 2e-2)
- GRU state as [128, 2n] combined-halves tiles; r and z share one PSUM tile
  and one Sigmoid (4n <= 512 everywhere after splitting facts into groups)
- the fact recurrence runs as 2 independent 80-sequence chains, interleaved,
  so each chain's elementwise latency hides under the other's matmuls
- episodic attention blend fused into the GRU tail:
  e' = e + g*(1-z)*(n - e)  (exact rewrite of blend(GRU))
- fc weights host-permuted so the packed [128, 2000] logit tile has a linear
  token map: token = halfv[p] + col; argmax is 3 full-width 16-bit DVE ops
"""

import os
import numpy as np
import ml_dtypes

import concourse.bass as bass
import concourse.bacc as bacc
import concourse.mybir as mybir
from concourse.tile import TileContext
from concourse.bass_utils import run_bass_kernel_spmd
from concourse.masks import make_identity

AF = mybir.ActivationFunctionType
ALU = mybir.AluOpType
DT = mybir.dt
BF16 = DT.bfloat16
F32 = DT.float32
F16 = DT.float16

V, E, H = 32000, 256, 256
B, NF, FL, QL = 64, 20, 32, 16
N_EPISODE = 3
SEQBEGIN = 1
NCORE = 8
BB = B // NCORE            # 8 batch rows per core
NSEQ = BB * NF             # 160 fact seqs per core
NTOK = NSEQ * FL           # 5120 fact tokens per core
VS = V // NCORE            # 4000 vocab shard
CHW = 500                  # fc chunk width (one PSUM half)
NPAIR = 4                  # 4 pairs x (2x500) = 4000
GF = 2                     # fact-recurrence groups
GS = NSEQ // GF            # 80 seqs per group

GK = {"ig": E, "qg": E, "att": H, "mem": H, "ans": 2 * H}


def build_nc(alen, fcb_nonzero):
    nc = bacc.Bacc("TRN2", num_devices=NCORE)

    def dram_in(name, shape, dtype=F32):
        return nc.dram_tensor(name, list(shape), dtype, kind="ExternalInput")

    io = {}
    io["facts_idx"] = dram_in("facts_idx", [NTOK, 1], DT.int32)
    io["q_idx"] = dram_in("q_idx", [BB * QL, 1], DT.int32)
    io["embed"] = dram_in("embed", [V, E], BF16)
    io["fcwT"] = dram_in("fcwT", [E, VS], BF16)
    io["last0T"] = dram_in("last0T", [E, B], BF16)
    io["offc2"] = dram_in("offc2", [128, NPAIR])
    if fcb_nonzero:
        io["fcbp"] = dram_in("fcbp", [128, NPAIR * CHW])
    for g, kin in GK.items():
        io[f"{g}_wihT"] = dram_in(f"{g}_wihT", [kin, 3 * H], BF16)
        io[f"{g}_whhT"] = dram_in(f"{g}_whhT", [H, 3 * H], BF16)
        io[f"{g}_brz"] = dram_in(f"{g}_brz", [128, 4])
        io[f"{g}_bin"] = dram_in(f"{g}_bin", [128, 2])
        io[f"{g}_bhn"] = dram_in(f"{g}_bhn", [128, 2])
        io[f"{g}_bhnr"] = dram_in(f"{g}_bhnr", [1, 256], BF16)
    io["g1T"] = dram_in("g1T", [4 * H, H], BF16)
    io["g2T"] = dram_in("g2T", [H, 1], BF16)
    io["gb1"] = dram_in("gb1", [128, 2])
    io["gb2"] = dram_in("gb2", [1, 1])

    out_logp = nc.dram_tensor("out_logp", [B * alen, VS], F32, kind="ExternalOutput")

    cc_warm_in = nc.dram_tensor("cc_warm_in", [8, 4], F32, kind="Internal")
    cc_warm_out = nc.dram_tensor("cc_warm_out", [NCORE * 8, 4], F32, kind="Internal",
                                 addr_space="Shared")
    cc_enc_in = nc.dram_tensor("cc_enc_in", [BB, 2 * H], F32, kind="Internal")
    cc_enc_out = nc.dram_tensor("cc_enc_out", [B, 2 * H], F32, kind="Internal",
                                addr_space="Shared")
    cc_top_in = [nc.dram_tensor(f"cc_top_in{t}", [128, 4], F32, kind="Internal")
                 for t in range(alen)]
    cc_top_out = [nc.dram_tensor(f"cc_top_out{t}", [NCORE * 128, 4], F32,
                                 kind="Internal", addr_space="Shared")
                  for t in range(alen)]
    rg = [list(range(NCORE))]

    with TileContext(nc) as tc:
        with tc.tile_pool(name="state", bufs=1) as st, \
             tc.tile_pool(name="work", bufs=2) as wk, \
             tc.tile_pool(name="ps", bufs=8, space="PSUM") as ps:

            ident = st.tile([128, 128], BF16)
            make_identity(nc, ident[:, :])
            identf = st.tile([128, 128], F32)
            make_identity(nc, identf[:, :])
            ones1 = st.tile([1, 128], BF16)
            nc.vector.memset(ones1[:, :], 1.0)
            zb = st.tile([128, 1], F32)
            nc.vector.memset(zb[:, :], 0.0)

            # ---- load all weights up front (DMA overlaps everything) ----
            W = {}
            for g, kin in GK.items():
                W[f"{g}_wihT"] = []
                for k in range(kin // 128):
                    t = st.tile([128, 3 * H], BF16, name=f"{g}wih{k}")
                    nc.sync.dma_start(t[:, :], io[f"{g}_wihT"][k * 128:(k + 1) * 128, :])
                    W[f"{g}_wihT"].append(t)
                W[f"{g}_whhT"] = []
                for k in range(2):
                    t = st.tile([128, 3 * H], BF16, name=f"{g}whh{k}")
                    nc.sync.dma_start(t[:, :], io[f"{g}_whhT"][k * 128:(k + 1) * 128, :])
                    W[f"{g}_whhT"].append(t)
                for bn, w in (("brz", 4), ("bin", 2), ("bhn", 2)):
                    t = st.tile([128, w], F32, name=f"{g}{bn}")
                    nc.sync.dma_start(t[:, :], io[f"{g}_{bn}"][:, :])
                    W[f"{g}_{bn}"] = t
                t = st.tile([1, 256], BF16, name=f"{g}bhnr")
                nc.sync.dma_start(t[:, :], io[f"{g}_bhnr"][:, :])
                W[f"{g}_bhnr"] = t
            g1T = []
            for k in range(8):
                t = st.tile([128, H], BF16, name=f"g1T{k}")
                nc.sync.dma_start(t[:, :], io["g1T"][k * 128:(k + 1) * 128, :])
                g1T.append(t)
            g2T = []
            for k in range(2):
                t = st.tile([128, 1], BF16, name=f"g2T{k}")
                nc.sync.dma_start(t[:, :], io["g2T"][k * 128:(k + 1) * 128, :])
                g2T.append(t)
            gb1 = st.tile([128, 2], F32)
            nc.sync.dma_start(gb1[:, :], io["gb1"][:, :])
            gb2 = st.tile([1, 1], F32)
            nc.sync.dma_start(gb2[:, :], io["gb2"][:, :])
            fcwT = []
            for k in range(2):
                t = st.tile([128, VS], BF16, name=f"fcwT{k}")
                nc.sync.dma_start(t[:, :], io["fcwT"][k * 128:(k + 1) * 128, :])
                fcwT.append(t)
            lastT0 = st.tile([128, 2 * B], BF16, name="lastT0")
            for k in range(2):
                nc.sync.dma_start(lastT0[:, k * B:(k + 1) * B],
                                  io["last0T"][k * 128:(k + 1) * 128, :])
            offc2 = st.tile([128, NPAIR], F32)
            nc.sync.dma_start(offc2[:, :], io["offc2"][:, :])
            if fcb_nonzero:
                fcbp = st.tile([128, NPAIR * CHW], F32)
                nc.sync.dma_start(fcbp[:, :], io["fcbp"][:, :])

            evict_rr = [0]

            def evict(dst_ap, src_ap, bias=None):
                """PSUM -> SBUF eviction, alternating DVE/ACT, optional
                per-partition bias add (bias: [128,1] AP)."""
                if bias is None:
                    if evict_rr[0] % 2 == 0:
                        nc.vector.tensor_copy(dst_ap, src_ap)
                    else:
                        nc.scalar.activation(dst_ap, src_ap, AF.Copy)
                else:
                    if evict_rr[0] % 2 == 0:
                        nc.vector.tensor_scalar(dst_ap, src_ap, bias, None, ALU.add)
                    else:
                        nc.scalar.activation(dst_ap, src_ap, AF.Identity,
                                             bias=bias, scale=1.0)
                evict_rr[0] += 1

            def v3(ap2d, n):
                return ap2d.rearrange("p (c n) -> p c n", c=2)

            def v4(ap2d, n):
                return ap2d.rearrange("p (c n) -> p c n", c=4)

            # ================= GRU cell, combined-halves, fused r+z ==========
            def gru2_hmm(g, h_ap, n, has_x, name=""):
                """h-side matmuls of a GRU step into fresh PSUM tiles: prz
                [128, 4n] (r,z gate quarters) + pnh [128, 2n] (+pni for x's
                n-gate when has_x). With has_x the r/z groups stay open."""
                hw = W[f"{g}_whhT"]
                prz = ps.tile([128, 4 * n], F32, tag="bank", name=f"{name}prz")
                pnh = ps.tile([128, 2 * n], F32, tag="bank", name=f"{name}pnh")
                pni = ps.tile([128, 2 * n], F32, tag="bank", name=f"{name}pni") \
                    if has_x else None
                bhnr = W[f"{g}_bhnr"]
                for m in range(6):
                    dd = prz[:, m * n:(m + 1) * n] if m < 4 else \
                        pnh[:, (m - 4) * n:(m - 3) * n]
                    # r/z groups close here unless x-matmuls continue them;
                    # the n-gate group closes with the bhn bias rank-1 matmul.
                    for k in range(2):
                        nc.tensor.matmul(dd, hw[k][:, m * 128:(m + 1) * 128],
                                         h_ap[:, k * n:(k + 1) * n],
                                         start=(k == 0),
                                         stop=(k == 1 and m < 4 and not has_x))
                    if m >= 4:
                        nc.tensor.matmul(dd, bhnr[0:1, (m - 4) * 128:(m - 3) * 128],
                                         ones1[0:1, 0:n], start=False, stop=True)
                return (prz, pnh, pni)

            def gru2_rest(g, P, h_ap, n, gi_rz, gi_n, rhs_x=None, xk=(0, 2),
                          att_g=None, name=""):
                """Finish a GRU step: x-side matmuls (if any) + elementwise.
                gi_rz: [128, 4, n] AP (x-proj r/z quarters, biases folded).
                gi_n: [128, 2, n] AP. att_g: optional [128, 2, n] gate AP g_i;
                when given, returns the fused episode update
                e + g*(1-z)*(tanh(u) - e) instead of the plain GRU output."""
                prz, pnh, pni = P
                if rhs_x is not None:
                    xw = W[f"{g}_wihT"][xk[0]:xk[1]]
                    for m in range(6):
                        dd = prz[:, m * n:(m + 1) * n] if m < 4 else \
                            pni[:, (m - 4) * n:(m - 3) * n]
                        for k in range(2):
                            nc.tensor.matmul(dd, xw[k][:, m * 128:(m + 1) * 128],
                                             rhs_x[:, k * n:(k + 1) * n],
                                             start=(m >= 4 and k == 0),
                                             stop=(k == 1))

                arz = wk.tile([128, 4 * n], BF16, tag=f"arz{n}", name=f"{name}arz")
                nc.vector.tensor_add(v4(arz[:, :], n), v4(prz[:, :], n), gi_rz)
                rz = wk.tile([128, 4 * n], BF16, tag=f"rz{n}", name=f"{name}rz")
                nc.scalar.activation(rz[:, :], arz[:, :], AF.Sigmoid, bias=zb[:, :],
                                     scale=1.0)
                r = rz[:, 0:2 * n]
                z = rz[:, 2 * n:4 * n]
                y = wk.tile([128, 2 * n], BF16, tag=f"y{n}", name=f"{name}y")
                nc.vector.tensor_mul(y[:, :], pnh[:, :], r)
                u = wk.tile([128, 2 * n], BF16, tag=f"u{n}", name=f"{name}u")
                if pni is not None:
                    t2 = wk.tile([128, 2 * n], BF16, tag=f"t2{n}", name=f"{name}t2")
                    nc.vector.tensor_add(v3(t2[:, :], n), v3(pni[:, :], n), gi_n)
                    nc.vector.tensor_add(u[:, :], y[:, :], t2[:, :])
                else:
                    nc.vector.tensor_add(v3(u[:, :], n), v3(y[:, :], n), gi_n)
                nn = wk.tile([128, 2 * n], BF16, tag=f"nn{n}", name=f"{name}nn")
                nc.scalar.activation(nn[:, :], u[:, :], AF.Tanh, bias=zb[:, :],
                                     scale=1.0)
                h2 = wk.tile([128, 2 * n], BF16, tag=f"h2{n}", bufs=4,
                             name=f"{name}h2")
                if att_g is None:
                    te = nc.vector
                    dt_ = wk.tile([128, 2 * n], BF16, tag=f"d{n}", name=f"{name}d")
                    te.tensor_sub(dt_[:, :], h_ap, nn[:, :])
                    w2 = wk.tile([128, 2 * n], BF16, tag=f"w2{n}", name=f"{name}w2")
                    te.tensor_mul(w2[:, :], z, dt_[:, :])
                    te.tensor_add(h2[:, :], nn[:, :], w2[:, :])
                else:
                    omz = wk.tile([128, 2 * n], BF16, tag=f"omz{n}",
                                  name=f"{name}omz")
                    nc.vector.tensor_scalar(omz[:, :], z, -1.0, 1.0, ALU.mult,
                                            ALU.add)
                    c2 = wk.tile([128, 2 * n], BF16, tag=f"c2{n}", name=f"{name}c2")
                    nc.vector.tensor_mul(v3(c2[:, :], n), v3(omz[:, :], n), att_g)
                    dd2 = wk.tile([128, 2 * n], BF16, tag=f"dd{n}", name=f"{name}dd")
                    nc.vector.tensor_sub(dd2[:, :], nn[:, :], h_ap)
                    p2 = wk.tile([128, 2 * n], BF16, tag=f"p2{n}", name=f"{name}p2")
                    nc.vector.tensor_mul(p2[:, :], c2[:, :], dd2[:, :])
                    nc.vector.tensor_add(h2[:, :], h_ap, p2[:, :])
                return h2

            def gru2(g, h_ap, n, gi_rz, gi_n, rhs_x=None, xk=(0, 2), att_g=None,
                     name=""):
                P = gru2_hmm(g, h_ap, n, rhs_x is not None, name=name)
                return gru2_rest(g, P, h_ap, n, gi_rz, gi_n, rhs_x=rhs_x, xk=xk,
                                 att_g=att_g, name=name)

            # helper: hoist an x-projection (all biases folded) into
            # girz [128, 4*N] + gin [128, 2*N] tiles, N tokens from rhs
            def hoist(g, rhs_fn, N, girz, gin, c0, c1, cw, tag):
                for c in range(c0, c1):
                    for m in range(6):
                        pp = ps.tile([128, cw], F32, tag="bank",
                                     name=f"{tag}xp{m}_{c}")
                        for k in range(2):
                            nc.tensor.matmul(pp[:, :],
                                             W[f"{g}_wihT"][k][:, m * 128:(m + 1) * 128],
                                             rhs_fn(k, c), start=(k == 0),
                                             stop=(k == 1))
                        if m < 4:
                            bias = W[f"{g}_brz"][:, m:m + 1]
                            dst = girz[:, m * N + c * cw:m * N + (c + 1) * cw]
                        else:
                            bias = W[f"{g}_bin"][:, m - 4:m - 3]
                            dst = gin[:, (m - 4) * N + c * cw:(m - 4) * N + (c + 1) * cw]
                        evict(dst, pp[:, :], bias)

            # ============ P1: gather facts + transpose + hoist x-proj ============
            fp_cm = tc.tile_pool(name="fpool", bufs=1)
            fp = fp_cm.__enter__()
            fidx = st.tile([128, NTOK // 128], DT.int32, name="fidx")
            nc.sync.dma_start(fidx[:, :], io["facts_idx"].rearrange(
                "(b a) o -> a (b o)", a=128))
            qidx = st.tile([128, 1], DT.int32, name="qidx")
            nc.sync.dma_start(qidx[:, :], io["q_idx"][:, :])

            # q gather FIRST so the question GRU can interleave from step 0
            qg_t = wk.tile([128, E], BF16, tag="fgat", bufs=4, name="qgat")
            nc.gpsimd.indirect_dma_start(
                out=qg_t[:, :], out_offset=None, in_=io["embed"][:, :],
                in_offset=bass.IndirectOffsetOnAxis(ap=qidx[:, :1], axis=0),
            )
            NQ = BB * QL
            XQT = fp.tile([128, 2 * NQ], BF16, name="XQT")
            for ch in range(2):
                pt = ps.tile([128, 128], BF16, tag="bank", name=f"qtp{ch}")
                nc.tensor.transpose(pt[:, :], qg_t[:, ch * 128:(ch + 1) * 128],
                                    ident[:, :])
                evict(XQT[:, ch * NQ:(ch + 1) * NQ], pt[:, :])
            girz_q = fp.tile([128, 4 * NQ], BF16, name="girz_q")
            gin_q = fp.tile([128, 2 * NQ], BF16, name="gin_q")
            hoist("qg", lambda k, c: XQT[:, k * NQ:(k + 1) * NQ], NQ,
                  girz_q[:, :], gin_q[:, :], 0, 1, NQ, "q")
            girz_qv = girz_q[:, :].rearrange("p (m t) -> p m t", m=4)
            gin_qv = gin_q[:, :].rearrange("p (m t) -> p m t", m=2)

            # ==== P1+P2 pipelined: per 640-token block, gather 5 embeddings,
            # hoist the block's x-projection, then run its 4 GRU steps —
            # the recurrence starts as soon as block 0 lands.  ====
            NB = FL // 4                       # 8 blocks of 4 steps
            BTOK = 640                         # tokens per block
            XTb = [[fp.tile([128, BTOK], BF16, name=f"XT{k}_{tb}")
                    for tb in range(NB)] for k in range(2)]
            girz_fb = [fp.tile([128, 4 * BTOK], BF16, name=f"girzf{tb}")
                       for tb in range(NB)]
            gin_fb = [fp.tile([128, 2 * BTOK], BF16, name=f"ginf{tb}")
                      for tb in range(NB)]

            hfs = []
            for gidx in range(GF):
                t = wk.tile([128, 2 * GS], BF16, tag="hfs", name=f"hf_init{gidx}")
                nc.vector.memset(t[:, :], 0.0)
                hfs.append(t[:, :])
            hq = wk.tile([128, 2 * BB], BF16, tag="hqs", name="hq_init")
            nc.vector.memset(hq[:, :], 0.0)
            hq_ap = hq[:, :]

            for tb in range(NB):
                for ii in range(5):
                    i = tb * 5 + ii
                    gt = wk.tile([128, E], BF16, tag="fgat", bufs=4, name=f"fg{i}")
                    nc.gpsimd.indirect_dma_start(
                        out=gt[:, :], out_offset=None, in_=io["embed"][:, :],
                        in_offset=bass.IndirectOffsetOnAxis(ap=fidx[:, i:i + 1],
                                                            axis=0),
                    )
                    for ch in range(2):
                        pt = ps.tile([128, 128], BF16, tag="bank",
                                     name=f"ftp{i}_{ch}")
                        nc.tensor.transpose(pt[:, :],
                                            gt[:, ch * 128:(ch + 1) * 128],
                                            ident[:, :])
                        evict(XTb[ch][tb][:, ii * 128:(ii + 1) * 128], pt[:, :])
                hoist("ig", lambda k, c: XTb[k][tb][:, c * 320:(c + 1) * 320],
                      BTOK, girz_fb[tb][:, :], gin_fb[tb][:, :], 0, 2, 320,
                      f"f{tb}")
                girz_bv = girz_fb[tb][:, :].rearrange("p (m t) -> p m t", m=4)
                gin_bv = gin_fb[tb][:, :].rearrange("p (m t) -> p m t", m=2)
                for s4 in range(4):
                    step = tb * 4 + s4
                    Ps = [gru2_hmm("ig", hfs[gidx], GS, False,
                                   name=f"f{step}g{gidx}_")
                          for gidx in range(GF)]
                    for gidx in range(GF):
                        base = s4 * NSEQ + gidx * GS
                        h2 = gru2_rest("ig", Ps[gidx], hfs[gidx], GS,
                                       girz_bv[:, :, base:base + GS],
                                       gin_bv[:, :, base:base + GS],
                                       name=f"f{step}g{gidx}_")
                        hfs[gidx] = h2[:, :]
                    if step % 2 == 0 and step // 2 < QL:
                        qs = step // 2
                        q2 = gru2("qg", hq_ap, BB,
                                  girz_qv[:, :, qs * BB:(qs + 1) * BB],
                                  gin_qv[:, :, qs * BB:(qs + 1) * BB],
                                  name=f"q{qs}_")
                        hq_ap = q2[:, :]

            # warmup collective: gpsimd queue is clear of gathers now; this
            # pays the first-collective setup well before the enc AllGather.
            nc.gpsimd.collective_compute("AllGather", ALU.bypass,
                                         ins=[cc_warm_in[:, :]],
                                         outs=[cc_warm_out[:, :]],
                                         replica_groups=rg)

            encfT = st.tile([128, 2 * NSEQ], BF16, name="encfT")
            encf_v = encfT[:, :].rearrange("p (h t) -> p h t", h=2)
            for gidx in range(GF):
                nc.vector.tensor_copy(encf_v[:, :, gidx * GS:(gidx + 1) * GS],
                                      v3(hfs[gidx], GS))
            hqT = st.tile([128, 2 * BB], BF16, name="hqT")
            nc.vector.tensor_copy(hqT[:, :], hq_ap)
            fp_cm.__exit__(None, None, None)

            # ================= P3: episodes =================
            girz_a = st.tile([128, 4 * NSEQ], BF16, name="girz_a")
            gin_a = st.tile([128, 2 * NSEQ], BF16, name="gin_a")
            hoist("att", lambda k, c: encfT[:, k * NSEQ:(k + 1) * NSEQ], NSEQ,
                  girz_a[:, :], gin_a[:, :], 0, 1, NSEQ, "a")
            girz_av = girz_a[:, :].rearrange("p (m b i) -> p m b i", m=4, i=NF)
            gin_av = gin_a[:, :].rearrange("p (m b i) -> p m b i", m=2, i=NF)

            mem_girz = W["mem_brz"][:, 0:4].to_broadcast([128, 4, BB])
            mem_gin = W["mem_bin"][:, 0:2].to_broadcast([128, 2, BB])

            memT = st.tile([128, 2 * BB], BF16, name="memT")
            nc.vector.tensor_copy(memT[:, :], hqT[:, :])
            mem_ap = memT[:, :]

            encfv = encfT[:, :].rearrange("p (h b i) -> p h b i", h=2, i=NF)
            qbv = hqT[:, :].rearrange("p (h b) -> p h b", h=2).to_broadcast(
                [128, 2, BB, NF])

            for ep in range(N_EPISODE):
                memv = mem_ap.rearrange("p (h b) -> p h b", h=2).to_broadcast(
                    [128, 2, BB, NF])
                ZT = [wk.tile([128, 2 * NSEQ], BF16, tag=f"zt{x}", bufs=2,
                              name=f"ZT{ep}_{x}") for x in range(4)]
                zv = [t[:, :].rearrange("p (h b i) -> p h b i", h=2, i=NF)
                      for t in ZT]
                nc.vector.tensor_mul(zv[0], encfv, qbv)
                nc.vector.tensor_mul(zv[1], encfv, memv)
                dq = wk.tile([128, 2 * NSEQ], BF16, tag="dq", name=f"dq{ep}")
                nc.vector.tensor_sub(dq[:, :].rearrange("p (h b i) -> p h b i",
                                                        h=2, i=NF), encfv, qbv)
                nc.scalar.activation(ZT[2][:, :], dq[:, :], AF.Abs)
                dm = wk.tile([128, 2 * NSEQ], BF16, tag="dm", name=f"dm{ep}")
                nc.vector.tensor_sub(dm[:, :].rearrange("p (h b i) -> p h b i",
                                                        h=2, i=NF), encfv, memv)
                nc.scalar.activation(ZT[3][:, :], dm[:, :], AF.Abs)

                p1T = []
                for m in range(2):
                    pp = ps.tile([128, NSEQ], F32, tag="bank", name=f"p1{ep}_{m}")
                    for kt in range(8):
                        nc.tensor.matmul(pp[:, :], g1T[kt][:, m * 128:(m + 1) * 128],
                                         ZT[kt // 2][:, (kt % 2) * NSEQ:
                                                     (kt % 2 + 1) * NSEQ],
                                         start=(kt == 0), stop=(kt == 7))
                    t1 = wk.tile([128, NSEQ], BF16, tag="p1s", bufs=2,
                                 name=f"p1s{ep}_{m}")
                    nc.scalar.activation(t1[:, :], pp[:, :], AF.Tanh,
                                         bias=gb1[:, m:m + 1], scale=1.0)
                    p1T.append(t1)
                pgp = ps.tile([1, NSEQ], F32, tag="bank", name=f"pg{ep}")
                for k in range(2):
                    nc.tensor.matmul(pgp[:, :], g2T[k][:, :], p1T[k][:, :],
                                     start=(k == 0), stop=(k == 1))
                g_row = wk.tile([1, NSEQ], BF16, tag="grow", name=f"grow{ep}")
                nc.scalar.activation(g_row[:, :], pgp[:, :], AF.Sigmoid,
                                     bias=gb2[:, :], scale=1.0)
                pgB = ps.tile([128, 2 * NSEQ], F32, tag="bank", name=f"pgB{ep}")
                nc.tensor.matmul(pgB[:, 0:NSEQ], ones1[:, :], g_row[:, :],
                                 start=True, stop=True)
                nc.tensor.matmul(pgB[:, NSEQ:2 * NSEQ], ones1[:, :], g_row[:, :],
                                 start=True, stop=True)
                gBc = wk.tile([128, 2 * NSEQ], BF16, tag="gBc", name=f"gBc{ep}")
                evict(gBc[:, :], pgB[:, :])
                gBv = gBc[:, :].rearrange("p (h b i) -> p h b i", h=2, i=NF)

                eT = wk.tile([128, 2 * BB], BF16, tag="eTs", name=f"eT{ep}")
                nc.vector.memset(eT[:, :], 0.0)
                e_ap = eT[:, :]
                for i in range(NF):
                    enew = gru2("att", e_ap, BB, girz_av[:, :, :, i],
                                gin_av[:, :, :, i], att_g=gBv[:, :, :, i],
                                name=f"e{ep}_{i}_")
                    e_ap = enew[:, :]
                mnew = gru2("mem", mem_ap, BB, mem_girz, mem_gin,
                            rhs_x=e_ap, xk=(0, 2), name=f"m{ep}_")
                mem_ap = mnew[:, :]

            memF = st.tile([128, 2 * BB], BF16, name="memF")
            nc.vector.tensor_copy(memF[:, :], mem_ap)

            # ================= P4: all-gather mem|enc_q =================
            encrow = wk.tile([BB, 2 * H], F32, name="encrow")
            for ch in range(2):
                pt = ps.tile([BB, 128], BF16, tag="bank", name=f"egm{ch}")
                nc.tensor.transpose(pt[:, :], memF[:, ch * BB:(ch + 1) * BB],
                                    ident[:, :])
                evict(encrow[:, ch * 128:(ch + 1) * 128], pt[:, :])
                pt2 = ps.tile([BB, 128], BF16, tag="bank", name=f"egq{ch}")
                nc.tensor.transpose(pt2[:, :], hqT[:, ch * BB:(ch + 1) * BB],
                                    ident[:, :])
                evict(encrow[:, 256 + ch * 128:256 + (ch + 1) * 128], pt2[:, :])
            nc.sync.dma_start(cc_enc_in[:, :], encrow[:, :])
            nc.gpsimd.collective_compute("AllGather", ALU.bypass,
                                         ins=[cc_enc_in[:, :]],
                                         outs=[cc_enc_out[:, :]], replica_groups=rg)
            enc_all = wk.tile([B, 2 * H], F32, name="enc_all")
            nc.sync.dma_start(enc_all[:, :], cc_enc_out[:, :])

            memA = st.tile([128, 2 * B], BF16, name="memA")
            qA = st.tile([128, 2 * B], BF16, name="qA")
            for ch in range(2):
                pt = ps.tile([128, B], F32, tag="bank", name=f"tmA{ch}")
                nc.tensor.transpose(pt[:, :], enc_all[:, ch * 128:(ch + 1) * 128],
                                    identf[:B, :B])
                evict(memA[:, ch * B:(ch + 1) * B], pt[:, :])
                pt2 = ps.tile([128, B], F32, tag="bank", name=f"tqA{ch}")
                nc.tensor.transpose(pt2[:, :], enc_all[:, 256 + ch * 128:
                                                       256 + (ch + 1) * 128],
                                    identf[:B, :B])
                evict(qA[:, ch * B:(ch + 1) * B], pt2[:, :])

            # ================= P5: decode =================
            iota_i = st.tile([128, CHW], DT.int32)
            nc.gpsimd.iota(iota_i[:, :], pattern=[[1, CHW]], base=0,
                           channel_multiplier=0)
            iota_h = st.tile([128, CHW], F16)
            nc.vector.tensor_copy(iota_h[:, :], iota_i[:, :])

            girz_A = st.tile([128, 4 * B], BF16, name="girz_A")
            gin_A = st.tile([128, 2 * B], BF16, name="gin_A")
            # hoist uses wihT tiles [0:2]; ans q-part lives in tiles [2:4]
            for m in range(6):
                pp = ps.tile([128, B], F32, tag="bank", name=f"dxp{m}")
                for k in range(2):
                    nc.tensor.matmul(pp[:, :],
                                     W["ans_wihT"][2 + k][:, m * 128:(m + 1) * 128],
                                     qA[:, k * B:(k + 1) * B],
                                     start=(k == 0), stop=(k == 1))
                if m < 4:
                    bias = W["ans_brz"][:, m:m + 1]
                    dst = girz_A[:, m * B:(m + 1) * B]
                else:
                    bias = W["ans_bin"][:, m - 4:m - 3]
                    dst = gin_A[:, (m - 4) * B:(m - 3) * B]
                evict(dst, pp[:, :], bias)
            girz_Av = girz_A[:, :].rearrange("p (m t) -> p m t", m=4)
            gin_Av = gin_A[:, :].rearrange("p (m t) -> p m t", m=2)

            hid_ap = memA[:, :]
            last_ap = lastT0[:, :]
            out3 = out_logp.rearrange("(b t) v -> b t v", t=alen)
            W2K = NPAIR * CHW  # 2000 packed cols per partition-half

            Pd = gru2_hmm("ans", hid_ap, B, True, name="a0_")
            for t_step in range(alen):
                h2 = gru2_rest("ans", Pd, hid_ap, B, girz_Av, gin_Av,
                               rhs_x=last_ap, xk=(0, 2), name=f"a{t_step}_")
                hid_ap = h2[:, :]

                lg = wk.tile([128, W2K], BF16, tag="lg", bufs=1,
                             name=f"lg{t_step}")
                sext = wk.tile([128, NPAIR], F32, tag="sext", name=f"sx{t_step}")
                Mt = wk.tile([128, NPAIR], BF16, tag="Mt", name=f"Mt{t_step}")
                It = wk.tile([128, NPAIR], F16, tag="It", name=f"It{t_step}")
                for j in range(NPAIR):
                    pl = ps.tile([128, CHW], F32, tag="bank", name=f"pl{t_step}_{j}")
                    for half in range(2):
                        oap = pl[half * B:(half + 1) * B, :]
                        for k in range(2):
                            nc.tensor.matmul(
                                oap, hid_ap[:, k * B:(k + 1) * B],
                                fcwT[k][:, (2 * j + half) * CHW:
                                         (2 * j + half + 1) * CHW],
                                start=(k == 0), stop=(k == 1),
                                tile_position=(0, 64) if half else None)
                    if fcb_nonzero:
                        nc.vector.tensor_add(pl[:, :], pl[:, :],
                                             fcbp[:, j * CHW:(j + 1) * CHW])
                    evict(lg[:, j * CHW:(j + 1) * CHW], pl[:, :])
                    dump = wk.tile([128, CHW], BF16, tag="dump", bufs=2,
                                   name=f"dump{t_step}_{j}")
                    nc.scalar.activation(dump[:, :], pl[:, :], AF.Exp,
                                         bias=zb[:, :], scale=1.0,
                                         accum_out=sext[:, j:j + 1])
                    # per-pair max on DVE; mask+index on GpSimd (both engines
                    # overlap the remaining pairs' matmuls)
                    nc.vector.tensor_reduce(Mt[:, j:j + 1],
                                            lg[:, j * CHW:(j + 1) * CHW],
                                            axis=mybir.AxisListType.X, op=ALU.max)
                    msk = wk.tile([128, CHW], F16, tag="msk", bufs=2,
                                  name=f"msk{t_step}_{j}")
                    nc.vector.scalar_tensor_tensor(msk[:, :],
                                                   lg[:, j * CHW:(j + 1) * CHW],
                                                   Mt[:, j:j + 1], iota_h[:, :],
                                                   ALU.is_equal, ALU.mult)
                    nc.vector.tensor_reduce(It[:, j:j + 1], msk[:, :],
                                            axis=mybir.AxisListType.X, op=ALU.max)

                c1 = wk.tile([128, 1], BF16, tag="c1", name=f"c1{t_step}")
                nc.vector.tensor_reduce(c1[:, :], Mt[:, :],
                                        axis=mybir.AxisListType.X, op=ALU.max)
                itf = wk.tile([128, NPAIR], F32, tag="itf", name=f"itf{t_step}")
                nc.vector.tensor_copy(itf[:, :], It[:, :])
                itg = wk.tile([128, NPAIR], F32, tag="itg", name=f"itg{t_step}")
                nc.vector.tensor_add(itg[:, :], itf[:, :], offc2[:, :])
                wix = wk.tile([128, NPAIR], F32, tag="wix", name=f"wix{t_step}")
                nc.vector.scalar_tensor_tensor(wix[:, :], Mt[:, :], c1[:, :],
                                               itg[:, :], ALU.is_equal, ALU.mult)
                pack = wk.tile([128, 4], F32, tag="pack", name=f"pack{t_step}")
                nc.vector.tensor_copy(pack[:, 0:1], c1[:, :])
                nc.vector.tensor_reduce(pack[:, 1:2], wix[:, :],
                                        axis=mybir.AxisListType.X, op=ALU.max)
                nc.vector.tensor_reduce(pack[:, 2:3], sext[:, :],
                                        axis=mybir.AxisListType.X, op=ALU.add)
                nc.vector.tensor_copy(pack[:, 3:4], zb[:, :])
                nc.sync.dma_start(cc_top_in[t_step][:, :], pack[:, :])
                nc.gpsimd.collective_compute("AllGather", ALU.bypass,
                                             ins=[cc_top_in[t_step][:, :]],
                                             outs=[cc_top_out[t_step][:, :]],
                                             replica_groups=rg)
                # next step's h-side matmuls run on the PE during the AG
                if t_step < alen - 1:
                    Pd = gru2_hmm("ans", hid_ap, B, True, name=f"a{t_step + 1}_")
                # readback: row = core*128 + half*64 + batch
                topall = wk.tile([B, 16 * 4], F32, tag="topall",
                                 name=f"topall{t_step}")
                nc.sync.dma_start(
                    topall[:, :].rearrange("b (c v) -> b c v", v=4),
                    cc_top_out[t_step].rearrange("(c h b) v -> b (c h) v",
                                                 h=2, b=B),
                )
                t3 = topall[:, :].rearrange("b (c v) -> b c v", v=4)
                gv = wk.tile([B, 1], F32, tag="gv", name=f"gv{t_step}")
                nc.vector.tensor_reduce(gv[:, :], t3[:, :, 0],
                                        axis=mybir.AxisListType.X, op=ALU.max)
                wtokf = wk.tile([B, 16], F32, tag="wtokf", name=f"wtokf{t_step}")
                nc.vector.scalar_tensor_tensor(wtokf[:, :], t3[:, :, 0], gv[:, :],
                                               t3[:, :, 1], ALU.is_equal, ALU.mult)
                wtok = wk.tile([B, 1], F32, tag="wtok", name=f"wtok{t_step}")
                nc.vector.tensor_reduce(wtok[:, :], wtokf[:, :],
                                        axis=mybir.AxisListType.X, op=ALU.max)
                stot = wk.tile([B, 1], F32, tag="stot", name=f"stot{t_step}")
                nc.vector.tensor_reduce(stot[:, :], t3[:, :, 2],
                                        axis=mybir.AxisListType.X, op=ALU.add)
                nlz = wk.tile([128, 1], F32, tag="nlz", name=f"nlz{t_step}")
                nc.scalar.activation(nlz[0:B, :], stot[:, :], AF.Ln,
                                     bias=zb[0:B, :], scale=1.0)
                nc.vector.tensor_scalar_mul(nlz[0:B, :], nlz[0:B, :], -1.0)
                nc.sync.dma_start(nlz[B:2 * B, :], nlz[0:B, :])
                for j in range(NPAIR):
                    og = wk.tile([128, CHW], F32, tag="og", bufs=2,
                                 name=f"og{t_step}_{j}")
                    if j % 2 == 0:
                        nc.scalar.activation(og[:, :], lg[:, j * CHW:(j + 1) * CHW],
                                             AF.Identity, bias=nlz[:, :], scale=1.0)
                    else:
                        nc.vector.tensor_scalar(og[:, :], lg[:, j * CHW:(j + 1) * CHW],
                                                nlz[:, :], None, ALU.add)
                    nc.sync.dma_start(out3[:, t_step, j * CHW:(j + 1) * CHW],
                                      og[0:B, :])
                    nc.sync.dma_start(
                        out3[:, t_step, 2000 + j * CHW:2000 + (j + 1) * CHW],
                        og[B:2 * B, :])
                if t_step == alen - 1:
                    continue
                wtoki = wk.tile([B, 1], DT.int32, tag="wtoki", name=f"wtoki{t_step}")
                nc.vector.tensor_copy(wtoki[:, :], wtok[:, :])
                lemb = wk.tile([B, E], BF16, tag="lemb", name=f"lemb{t_step}")
                nc.gpsimd.indirect_dma_start(
                    out=lemb[:, :], out_offset=None, in_=io["embed"][:, :],
                    in_offset=bass.IndirectOffsetOnAxis(ap=wtoki[:, :1], axis=0),
                )
                newlast = wk.tile([128, 2 * B], BF16, tag="lastT", bufs=2,
                                  name=f"lastT{t_step}")
                for ch in range(2):
                    pt = ps.tile([128, B], BF16, tag="bank", name=f"lt{t_step}_{ch}")
                    nc.tensor.transpose(pt[:, :], lemb[:, ch * 128:(ch + 1) * 128],
                                        ident[:B, :B])
                    evict(newlast[:, ch * B:(ch + 1) * B], pt[:, :])
                last_ap = newlast[:, :]

    nc.finalize()
    return nc


def prep_inputs(inputs):
    """Host-side shard/pack. Returns in_maps list for the 8 cores."""
    f32 = np.float32
    bf16 = ml_dtypes.bfloat16
    emb = np.ascontiguousarray(np.asarray(inputs["embed_w"], f32).astype(bf16))
    packs = {}
    for g in GK:
        wih = np.asarray(inputs[f"{g}_wih"], dtype=f32)
        whh = np.asarray(inputs[f"{g}_whh"], dtype=f32)
        bih = np.asarray(inputs[f"{g}_bih"], dtype=f32)
        bhh = np.asarray(inputs[f"{g}_bhh"], dtype=f32)
        brz = bih[:512] + bhh[:512]
        packs[f"{g}_wihT"] = np.ascontiguousarray(wih.T.astype(bf16))
        packs[f"{g}_whhT"] = np.ascontiguousarray(whh.T.astype(bf16))
        packs[f"{g}_brz"] = np.ascontiguousarray(brz.reshape(4, 128).T)
        packs[f"{g}_bin"] = np.ascontiguousarray(bih[512:768].reshape(2, 128).T)
        packs[f"{g}_bhn"] = np.ascontiguousarray(bhh[512:768].reshape(2, 128).T)
        packs[f"{g}_bhnr"] = np.ascontiguousarray(
            bhh[512:768].reshape(1, 256).astype(bf16))
    packs["g1T"] = np.ascontiguousarray(np.asarray(inputs["gate_w1"], f32).T.astype(bf16))
    packs["g2T"] = np.ascontiguousarray(np.asarray(inputs["gate_w2"], f32).T.astype(bf16))
    packs["gb1"] = np.ascontiguousarray(np.asarray(inputs["gate_b1"], f32).reshape(2, 128).T)
    packs["gb2"] = np.asarray(inputs["gate_b2"], f32).reshape(1, 1)
    fcwT = np.ascontiguousarray(np.asarray(inputs["fc_w"], f32).T.astype(bf16))
    fcb = np.asarray(inputs["fc_b"], f32)
    fcb_nonzero = bool(np.any(fcb != 0))
    last0T = np.ascontiguousarray(
        np.tile(np.asarray(inputs["embed_w"], f32)[SEQBEGIN][:, None],
                (1, B)).astype(bf16))
    allfacts = np.asarray(inputs["allfacts"], np.int32)
    questions = np.asarray(inputs["questions"], np.int32)

    in_maps = []
    for j in range(NCORE):
        m = dict(packs)
        m["embed"] = emb
        # permute the vocab shard so packed col c on partition-half h maps to
        # token j*VS + h*2000 + c: new[:, jj*1000 + h*500 + r] = old h*2000+jj*500+r
        shard = fcwT[:, j * VS:(j + 1) * VS]
        sh3 = shard.reshape(E, 2, NPAIR, CHW)          # (h, jj, r)
        m["fcwT"] = np.ascontiguousarray(
            sh3.transpose(0, 2, 1, 3).reshape(E, VS))  # (jj, h, r)
        oc = np.zeros((128, NPAIR), f32)
        for jj in range(NPAIR):
            oc[0:B, jj] = j * VS + jj * CHW
            oc[B:128, jj] = j * VS + 2000 + jj * CHW
        m["offc2"] = oc
        if fcb_nonzero:
            fcbp = np.zeros((128, NPAIR * CHW), f32)
            sh = fcb[j * VS:(j + 1) * VS]
            for jj in range(NPAIR):
                fcbp[0:B, jj * CHW:(jj + 1) * CHW] = sh[jj * CHW:(jj + 1) * CHW]
                fcbp[B:128, jj * CHW:(jj + 1) * CHW] = \
                    sh[2000 + jj * CHW:2000 + (jj + 1) * CHW]
            m["fcbp"] = fcbp
        m["last0T"] = last0T
        m["facts_idx"] = np.ascontiguousarray(
            allfacts[j * BB:(j + 1) * BB].reshape(NSEQ, FL).T.reshape(-1, 1))
        m["q_idx"] = np.ascontiguousarray(
            questions[j * BB:(j + 1) * BB].reshape(BB, QL).T.reshape(-1, 1))
        in_maps.append(m)
    return in_maps, fcb_nonzero


_CACHE = {}


def kernel(**inputs):
    alen = int(inputs["alen"])
    in_maps, fcb_nonzero = prep_inputs(inputs)
    key = (alen, fcb_nonzero)
    if key not in _CACHE:
        _CACHE[key] = build_nc(alen, fcb_nonzero)
    nc = _CACHE[key]
    res = run_bass_kernel_spmd(nc, in_maps, core_ids=list(range(NCORE)))
    out = np.concatenate([res.results[j]["out_logp"] for j in range(NCORE)], axis=1)
    return out.astype(np.float32)


# revision 27
# speedup vs baseline: 1.0110x; 1.0110x over previous
"""DMN forward on 8 Trainium2 NeuronCores (Bass/Tile), v3 (bf16).

Sharding: batch rows 8/core for fact+question encoding and episodic memory;
decode GRU replicated on all cores, fc/log-softmax vocab-sharded 4000
columns/core. One tiny AllGather per decode step carries per-half
(max, argmax-token, sumexp); a dummy warmup AllGather early in the kernel
absorbs the first-collective latency while the fact GRU runs.

v3 structure:
- all matmuls bf16 (host-validated: relnorm ~1e-3 vs fp32 ref, gate 2e-2)
- GRU state as [128, 2n] combined-halves tiles; r and z share one PSUM tile
  and one Sigmoid (4n <= 512 everywhere after splitting facts into groups)
- the fact recurrence runs as 2 independent 80-sequence chains, interleaved,
  so each chain's elementwise latency hides under the other's matmuls
- episodic attention blend fused into the GRU tail:
  e' = e + g*(1-z)*(n - e)  (exact rewrite of blend(GRU))
- fc weights host-permuted so the packed [128, 2000] logit tile has a linear
  token map: token = halfv[p] + col; argmax is 3 full-width 16-bit DVE ops
"""

import os
import numpy as np
import ml_dtypes

import concourse.bass as bass
import concourse.bacc as bacc
import concourse.mybir as mybir
from concourse.tile import TileContext
from concourse.bass_utils import run_bass_kernel_spmd
from concourse.masks import make_identity

AF = mybir.ActivationFunctionType
ALU = mybir.AluOpType
DT = mybir.dt
BF16 = DT.bfloat16
F32 = DT.float32
F16 = DT.float16

V, E, H = 32000, 256, 256
B, NF, FL, QL = 64, 20, 32, 16
N_EPISODE = 3
SEQBEGIN = 1
NCORE = 8
BB = B // NCORE            # 8 batch rows per core
NSEQ = BB * NF             # 160 fact seqs per core
NTOK = NSEQ * FL           # 5120 fact tokens per core
VS = V // NCORE            # 4000 vocab shard
CHW = 500                  # fc chunk width (one PSUM half)
NPAIR = 4                  # 4 pairs x (2x500) = 4000
GF = 2                     # fact-recurrence groups
GS = NSEQ // GF            # 80 seqs per group

GK = {"ig": E, "qg": E, "att": H, "mem": H, "ans": 2 * H}


def build_nc(alen, fcb_nonzero):
    nc = bacc.Bacc("TRN2", num_devices=NCORE)

    def dram_in(name, shape, dtype=F32):
        return nc.dram_tensor(name, list(shape), dtype, kind="ExternalInput")

    io = {}
    io["facts_idx"] = dram_in("facts_idx", [NTOK, 1], DT.int32)
    io["q_idx"] = dram_in("q_idx", [BB * QL, 1], DT.int32)
    io["embed"] = dram_in("embed", [V, E], BF16)
    io["fcwT"] = dram_in("fcwT", [E, VS], BF16)
    io["last0T"] = dram_in("last0T", [E, B], BF16)
    io["offc2"] = dram_in("offc2", [128, NPAIR])
    if fcb_nonzero:
        io["fcbp"] = dram_in("fcbp", [128, NPAIR * CHW])
    for g, kin in GK.items():
        io[f"{g}_wihT"] = dram_in(f"{g}_wihT", [kin, 3 * H], BF16)
        io[f"{g}_whhT"] = dram_in(f"{g}_whhT", [H, 3 * H], BF16)
        io[f"{g}_brz"] = dram_in(f"{g}_brz", [128, 4])
        io[f"{g}_bin"] = dram_in(f"{g}_bin", [128, 2])
        io[f"{g}_bhn"] = dram_in(f"{g}_bhn", [128, 2])
        io[f"{g}_bhnr"] = dram_in(f"{g}_bhnr", [1, 256], BF16)
    io["g1T"] = dram_in("g1T", [4 * H, H], BF16)
    io["g2T"] = dram_in("g2T", [H, 1], BF16)
    io["gb1"] = dram_in("gb1", [128, 2])
    io["gb2"] = dram_in("gb2", [1, 1])

    out_logp = nc.dram_tensor("out_logp", [B * alen, VS], F32, kind="ExternalOutput")

    cc_warm_in = nc.dram_tensor("cc_warm_in", [8, 4], F32, kind="Internal")
    cc_warm_out = nc.dram_tensor("cc_warm_out", [NCORE * 8, 4], F32, kind="Internal",
                                 addr_space="Shared")
    cc_enc_in = nc.dram_tensor("cc_enc_in", [BB, 2 * H], F32, kind="Internal")
    cc_enc_out = nc.dram_tensor("cc_enc_out", [B, 2 * H], F32, kind="Internal",
                                addr_space="Shared")
    cc_top_in = [nc.dram_tensor(f"cc_top_in{t}", [128, 4], F32, kind="Internal")
                 for t in range(alen)]
    cc_top_out = [nc.dram_tensor(f"cc_top_out{t}", [NCORE * 128, 4], F32,
                                 kind="Internal", addr_space="Shared")
                  for t in range(alen)]
    rg = [list(range(NCORE))]

    with TileContext(nc) as tc:
        with tc.tile_pool(name="state", bufs=1) as st, \
             tc.tile_pool(name="work", bufs=2) as wk, \
             tc.tile_pool(name="ps", bufs=8, space="PSUM") as ps:

            ident = st.tile([128, 128], BF16)
            make_identity(nc, ident[:, :])
            identf = st.tile([128, 128], F32)
            make_identity(nc, identf[:, :])
            ones1 = st.tile([1, 128], BF16)
            nc.vector.memset(ones1[:, :], 1.0)
            zb = st.tile([128, 1], F32)
            nc.vector.memset(zb[:, :], 0.0)

            # ---- load all weights up front (DMA overlaps everything) ----
            W = {}
            for g, kin in GK.items():
                W[f"{g}_wihT"] = []
                for k in range(kin // 128):
                    t = st.tile([128, 3 * H], BF16, name=f"{g}wih{k}")
                    nc.sync.dma_start(t[:, :], io[f"{g}_wihT"][k * 128:(k + 1) * 128, :])
                    W[f"{g}_wihT"].append(t)
                W[f"{g}_whhT"] = []
                for k in range(2):
                    t = st.tile([128, 3 * H], BF16, name=f"{g}whh{k}")
                    nc.sync.dma_start(t[:, :], io[f"{g}_whhT"][k * 128:(k + 1) * 128, :])
                    W[f"{g}_whhT"].append(t)
                for bn, w in (("brz", 4), ("bin", 2), ("bhn", 2)):
                    t = st.tile([128, w], F32, name=f"{g}{bn}")
                    nc.sync.dma_start(t[:, :], io[f"{g}_{bn}"][:, :])
                    W[f"{g}_{bn}"] = t
                t = st.tile([1, 256], BF16, name=f"{g}bhnr")
                nc.sync.dma_start(t[:, :], io[f"{g}_bhnr"][:, :])
                W[f"{g}_bhnr"] = t
            g1T = []
            for k in range(8):
                t = st.tile([128, H], BF16, name=f"g1T{k}")
                nc.sync.dma_start(t[:, :], io["g1T"][k * 128:(k + 1) * 128, :])
                g1T.append(t)
            g2T = []
            for k in range(2):
                t = st.tile([128, 1], BF16, name=f"g2T{k}")
                nc.sync.dma_start(t[:, :], io["g2T"][k * 128:(k + 1) * 128, :])
                g2T.append(t)
            gb1 = st.tile([128, 2], F32)
            nc.sync.dma_start(gb1[:, :], io["gb1"][:, :])
            gb2 = st.tile([1, 1], F32)
            nc.sync.dma_start(gb2[:, :], io["gb2"][:, :])
            fcwT = []
            for k in range(2):
                t = st.tile([128, VS], BF16, name=f"fcwT{k}")
                nc.sync.dma_start(t[:, :], io["fcwT"][k * 128:(k + 1) * 128, :])
                fcwT.append(t)
            lastT0 = st.tile([128, 2 * B], BF16, name="lastT0")
            for k in range(2):
                nc.sync.dma_start(lastT0[:, k * B:(k + 1) * B],
                                  io["last0T"][k * 128:(k + 1) * 128, :])
            offc2 = st.tile([128, NPAIR], F32)
            nc.sync.dma_start(offc2[:, :], io["offc2"][:, :])
            if fcb_nonzero:
                fcbp = st.tile([128, NPAIR * CHW], F32)
                nc.sync.dma_start(fcbp[:, :], io["fcbp"][:, :])

            evict_rr = [0]

            def evict(dst_ap, src_ap, bias=None):
                """PSUM -> SBUF eviction, alternating DVE/ACT, optional
                per-partition bias add (bias: [128,1] AP)."""
                if bias is None:
                    if evict_rr[0] % 2 == 0:
                        nc.vector.tensor_copy(dst_ap, src_ap)
                    else:
                        nc.scalar.activation(dst_ap, src_ap, AF.Copy)
                else:
                    if evict_rr[0] % 2 == 0:
                        nc.vector.tensor_scalar(dst_ap, src_ap, bias, None, ALU.add)
                    else:
                        nc.scalar.activation(dst_ap, src_ap, AF.Identity,
                                             bias=bias, scale=1.0)
                evict_rr[0] += 1

            def v3(ap2d, n):
                return ap2d.rearrange("p (c n) -> p c n", c=2)

            def v4(ap2d, n):
                return ap2d.rearrange("p (c n) -> p c n", c=4)

            # ================= GRU cell, combined-halves, fused r+z ==========
            def gru2_hmm(g, h_ap, n, has_x, name=""):
                """h-side matmuls of a GRU step into fresh PSUM tiles: prz
                [128, 4n] (r,z gate quarters) + pnh [128, 2n] (+pni for x's
                n-gate when has_x). With has_x the r/z groups stay open."""
                hw = W[f"{g}_whhT"]
                prz = ps.tile([128, 4 * n], F32, tag="bank", name=f"{name}prz")
                pnh = ps.tile([128, 2 * n], F32, tag="bank", name=f"{name}pnh")
                pni = ps.tile([128, 2 * n], F32, tag="bank", name=f"{name}pni") \
                    if has_x else None
                bhnr = W[f"{g}_bhnr"]
                for m in range(6):
                    dd = prz[:, m * n:(m + 1) * n] if m < 4 else \
                        pnh[:, (m - 4) * n:(m - 3) * n]
                    # r/z groups close here unless x-matmuls continue them;
                    # the n-gate group closes with the bhn bias rank-1 matmul.
                    for k in range(2):
                        nc.tensor.matmul(dd, hw[k][:, m * 128:(m + 1) * 128],
                                         h_ap[:, k * n:(k + 1) * n],
                                         start=(k == 0),
                                         stop=(k == 1 and m < 4 and not has_x))
                    if m >= 4:
                        nc.tensor.matmul(dd, bhnr[0:1, (m - 4) * 128:(m - 3) * 128],
                                         ones1[0:1, 0:n], start=False, stop=True)
                return (prz, pnh, pni)

            def gru2_rest(g, P, h_ap, n, gi_rz, gi_n, rhs_x=None, xk=(0, 2),
                          att_g=None, name=""):
                """Finish a GRU step: x-side matmuls (if any) + elementwise.
                gi_rz: [128, 4, n] AP (x-proj r/z quarters, biases folded).
                gi_n: [128, 2, n] AP. att_g: optional [128, 2, n] gate AP g_i;
                when given, returns the fused episode update
                e + g*(1-z)*(tanh(u) - e) instead of the plain GRU output."""
                prz, pnh, pni = P
                if rhs_x is not None:
                    xw = W[f"{g}_wihT"][xk[0]:xk[1]]
                    for m in range(6):
                        dd = prz[:, m * n:(m + 1) * n] if m < 4 else \
                            pni[:, (m - 4) * n:(m - 3) * n]
                        for k in range(2):
                            nc.tensor.matmul(dd, xw[k][:, m * 128:(m + 1) * 128],
                                             rhs_x[:, k * n:(k + 1) * n],
                                             start=(m >= 4 and k == 0),
                                             stop=(k == 1))

                arz = wk.tile([128, 4 * n], BF16, tag=f"arz{n}", name=f"{name}arz")
                nc.vector.tensor_add(v4(arz[:, :], n), v4(prz[:, :], n), gi_rz)
                rz = wk.tile([128, 4 * n], BF16, tag=f"rz{n}", name=f"{name}rz")
                nc.scalar.activation(rz[:, :], arz[:, :], AF.Sigmoid, bias=zb[:, :],
                                     scale=1.0)
                r = rz[:, 0:2 * n]
                z = rz[:, 2 * n:4 * n]
                y = wk.tile([128, 2 * n], BF16, tag=f"y{n}", name=f"{name}y")
                nc.vector.tensor_mul(y[:, :], pnh[:, :], r)
                u = wk.tile([128, 2 * n], BF16, tag=f"u{n}", name=f"{name}u")
                if pni is not None:
                    t2 = wk.tile([128, 2 * n], BF16, tag=f"t2{n}", name=f"{name}t2")
                    nc.vector.tensor_add(v3(t2[:, :], n), v3(pni[:, :], n), gi_n)
                    nc.vector.tensor_add(u[:, :], y[:, :], t2[:, :])
                else:
                    nc.vector.tensor_add(v3(u[:, :], n), v3(y[:, :], n), gi_n)
                if att_g is not None:
                    # off-chain prep: only needs z and the (static) gate —
                    # emitted before the Tanh so it overlaps it
                    omz = wk.tile([128, 2 * n], BF16, tag=f"omz{n}",
                                  name=f"{name}omz")
                    nc.vector.tensor_scalar(omz[:, :], z, -1.0, 1.0, ALU.mult,
                                            ALU.add)
                    c2 = wk.tile([128, 2 * n], BF16, tag=f"c2{n}", name=f"{name}c2")
                    nc.vector.tensor_mul(v3(c2[:, :], n), v3(omz[:, :], n), att_g)
                nn = wk.tile([128, 2 * n], BF16, tag=f"nn{n}", name=f"{name}nn")
                nc.scalar.activation(nn[:, :], u[:, :], AF.Tanh, bias=zb[:, :],
                                     scale=1.0)
                h2 = wk.tile([128, 2 * n], BF16, tag=f"h2{n}", bufs=4,
                             name=f"{name}h2")
                if att_g is None:
                    dt_ = wk.tile([128, 2 * n], BF16, tag=f"d{n}", name=f"{name}d")
                    nc.vector.tensor_sub(dt_[:, :], h_ap, nn[:, :])
                    w2 = wk.tile([128, 2 * n], BF16, tag=f"w2{n}", name=f"{name}w2")
                    nc.vector.tensor_mul(w2[:, :], z, dt_[:, :])
                    nc.vector.tensor_add(h2[:, :], nn[:, :], w2[:, :])
                else:
                    dd2 = wk.tile([128, 2 * n], BF16, tag=f"dd{n}", name=f"{name}dd")
                    nc.vector.tensor_sub(dd2[:, :], nn[:, :], h_ap)
                    p2 = wk.tile([128, 2 * n], BF16, tag=f"p2{n}", name=f"{name}p2")
                    nc.vector.tensor_mul(p2[:, :], c2[:, :], dd2[:, :])
                    nc.vector.tensor_add(h2[:, :], h_ap, p2[:, :])
                return h2

            def gru2(g, h_ap, n, gi_rz, gi_n, rhs_x=None, xk=(0, 2), att_g=None,
                     name=""):
                P = gru2_hmm(g, h_ap, n, rhs_x is not None, name=name)
                return gru2_rest(g, P, h_ap, n, gi_rz, gi_n, rhs_x=rhs_x, xk=xk,
                                 att_g=att_g, name=name)

            # helper: hoist an x-projection (all biases folded) into
            # girz [128, 4*N] + gin [128, 2*N] tiles, N tokens from rhs
            def hoist(g, rhs_fn, N, girz, gin, c0, c1, cw, tag):
                for c in range(c0, c1):
                    for m in range(6):
                        pp = ps.tile([128, cw], F32, tag="bank",
                                     name=f"{tag}xp{m}_{c}")
                        for k in range(2):
                            nc.tensor.matmul(pp[:, :],
                                             W[f"{g}_wihT"][k][:, m * 128:(m + 1) * 128],
                                             rhs_fn(k, c), start=(k == 0),
                                             stop=(k == 1))
                        if m < 4:
                            bias = W[f"{g}_brz"][:, m:m + 1]
                            dst = girz[:, m * N + c * cw:m * N + (c + 1) * cw]
                        else:
                            bias = W[f"{g}_bin"][:, m - 4:m - 3]
                            dst = gin[:, (m - 4) * N + c * cw:(m - 4) * N + (c + 1) * cw]
                        evict(dst, pp[:, :], bias)

            # ============ P1: gather facts + transpose + hoist x-proj ============
            fp_cm = tc.tile_pool(name="fpool", bufs=1)
            fp = fp_cm.__enter__()
            fidx = st.tile([128, NTOK // 128], DT.int32, name="fidx")
            nc.sync.dma_start(fidx[:, :], io["facts_idx"].rearrange(
                "(b a) o -> a (b o)", a=128))
            qidx = st.tile([128, 1], DT.int32, name="qidx")
            nc.sync.dma_start(qidx[:, :], io["q_idx"][:, :])

            # q gather FIRST so the question GRU can interleave from step 0
            qg_t = wk.tile([128, E], BF16, tag="fgat", bufs=4, name="qgat")
            nc.gpsimd.indirect_dma_start(
                out=qg_t[:, :], out_offset=None, in_=io["embed"][:, :],
                in_offset=bass.IndirectOffsetOnAxis(ap=qidx[:, :1], axis=0),
            )
            NQ = BB * QL
            XQT = fp.tile([128, 2 * NQ], BF16, name="XQT")
            for ch in range(2):
                pt = ps.tile([128, 128], BF16, tag="bank", name=f"qtp{ch}")
                nc.tensor.transpose(pt[:, :], qg_t[:, ch * 128:(ch + 1) * 128],
                                    ident[:, :])
                evict(XQT[:, ch * NQ:(ch + 1) * NQ], pt[:, :])
            girz_q = fp.tile([128, 4 * NQ], BF16, name="girz_q")
            gin_q = fp.tile([128, 2 * NQ], BF16, name="gin_q")
            hoist("qg", lambda k, c: XQT[:, k * NQ:(k + 1) * NQ], NQ,
                  girz_q[:, :], gin_q[:, :], 0, 1, NQ, "q")
            girz_qv = girz_q[:, :].rearrange("p (m t) -> p m t", m=4)
            gin_qv = gin_q[:, :].rearrange("p (m t) -> p m t", m=2)

            # ==== P1+P2 pipelined: per 640-token block, gather 5 embeddings,
            # hoist the block's x-projection, then run its 4 GRU steps —
            # the recurrence starts as soon as block 0 lands.  ====
            NB = FL // 4                       # 8 blocks of 4 steps
            BTOK = 640                         # tokens per block
            XTb = [[fp.tile([128, BTOK], BF16, name=f"XT{k}_{tb}")
                    for tb in range(NB)] for k in range(2)]
            girz_fb = [fp.tile([128, 4 * BTOK], BF16, name=f"girzf{tb}")
                       for tb in range(NB)]
            gin_fb = [fp.tile([128, 2 * BTOK], BF16, name=f"ginf{tb}")
                      for tb in range(NB)]

            hfs = []
            for gidx in range(GF):
                t = wk.tile([128, 2 * GS], BF16, tag="hfs", name=f"hf_init{gidx}")
                nc.vector.memset(t[:, :], 0.0)
                hfs.append(t[:, :])
            hq = wk.tile([128, 2 * BB], BF16, tag="hqs", name="hq_init")
            nc.vector.memset(hq[:, :], 0.0)
            hq_ap = hq[:, :]

            for tb in range(NB):
                for ii in range(5):
                    i = tb * 5 + ii
                    gt = wk.tile([128, E], BF16, tag="fgat", bufs=4, name=f"fg{i}")
                    nc.gpsimd.indirect_dma_start(
                        out=gt[:, :], out_offset=None, in_=io["embed"][:, :],
                        in_offset=bass.IndirectOffsetOnAxis(ap=fidx[:, i:i + 1],
                                                            axis=0),
                    )
                    for ch in range(2):
                        pt = ps.tile([128, 128], BF16, tag="bank",
                                     name=f"ftp{i}_{ch}")
                        nc.tensor.transpose(pt[:, :],
                                            gt[:, ch * 128:(ch + 1) * 128],
                                            ident[:, :])
                        evict(XTb[ch][tb][:, ii * 128:(ii + 1) * 128], pt[:, :])
                hoist("ig", lambda k, c: XTb[k][tb][:, c * 320:(c + 1) * 320],
                      BTOK, girz_fb[tb][:, :], gin_fb[tb][:, :], 0, 2, 320,
                      f"f{tb}")
                girz_bv = girz_fb[tb][:, :].rearrange("p (m t) -> p m t", m=4)
                gin_bv = gin_fb[tb][:, :].rearrange("p (m t) -> p m t", m=2)
                for s4 in range(4):
                    step = tb * 4 + s4
                    Ps = [gru2_hmm("ig", hfs[gidx], GS, False,
                                   name=f"f{step}g{gidx}_")
                          for gidx in range(GF)]
                    for gidx in range(GF):
                        base = s4 * NSEQ + gidx * GS
                        h2 = gru2_rest("ig", Ps[gidx], hfs[gidx], GS,
                                       girz_bv[:, :, base:base + GS],
                                       gin_bv[:, :, base:base + GS],
                                       name=f"f{step}g{gidx}_")
                        hfs[gidx] = h2[:, :]
                    if step % 2 == 0 and step // 2 < QL:
                        qs = step // 2
                        q2 = gru2("qg", hq_ap, BB,
                                  girz_qv[:, :, qs * BB:(qs + 1) * BB],
                                  gin_qv[:, :, qs * BB:(qs + 1) * BB],
                                  name=f"q{qs}_")
                        hq_ap = q2[:, :]

            # warmup collective: gpsimd queue is clear of gathers now; this
            # pays the first-collective setup well before the enc AllGather.
            nc.gpsimd.collective_compute("AllGather", ALU.bypass,
                                         ins=[cc_warm_in[:, :]],
                                         outs=[cc_warm_out[:, :]],
                                         replica_groups=rg)

            encfT = st.tile([128, 2 * NSEQ], BF16, name="encfT")
            encf_v = encfT[:, :].rearrange("p (h t) -> p h t", h=2)
            for gidx in range(GF):
                nc.vector.tensor_copy(encf_v[:, :, gidx * GS:(gidx + 1) * GS],
                                      v3(hfs[gidx], GS))
            hqT = st.tile([128, 2 * BB], BF16, name="hqT")
            nc.vector.tensor_copy(hqT[:, :], hq_ap)
            fp_cm.__exit__(None, None, None)

            # ================= P3: episodes =================
            girz_a = st.tile([128, 4 * NSEQ], BF16, name="girz_a")
            gin_a = st.tile([128, 2 * NSEQ], BF16, name="gin_a")
            hoist("att", lambda k, c: encfT[:, k * NSEQ:(k + 1) * NSEQ], NSEQ,
                  girz_a[:, :], gin_a[:, :], 0, 1, NSEQ, "a")
            girz_av = girz_a[:, :].rearrange("p (m b i) -> p m b i", m=4, i=NF)
            gin_av = gin_a[:, :].rearrange("p (m b i) -> p m b i", m=2, i=NF)

            mem_girz = W["mem_brz"][:, 0:4].to_broadcast([128, 4, BB])
            mem_gin = W["mem_bin"][:, 0:2].to_broadcast([128, 2, BB])

            memT = st.tile([128, 2 * BB], BF16, name="memT")
            nc.vector.tensor_copy(memT[:, :], hqT[:, :])
            mem_ap = memT[:, :]

            encfv = encfT[:, :].rearrange("p (h b i) -> p h b i", h=2, i=NF)
            qbv = hqT[:, :].rearrange("p (h b) -> p h b", h=2).to_broadcast(
                [128, 2, BB, NF])

            for ep in range(N_EPISODE):
                memv = mem_ap.rearrange("p (h b) -> p h b", h=2).to_broadcast(
                    [128, 2, BB, NF])
                ZT = [wk.tile([128, 2 * NSEQ], BF16, tag=f"zt{x}", bufs=2,
                              name=f"ZT{ep}_{x}") for x in range(4)]
                zv = [t[:, :].rearrange("p (h b i) -> p h b i", h=2, i=NF)
                      for t in ZT]
                nc.vector.tensor_mul(zv[0], encfv, qbv)
                nc.vector.tensor_mul(zv[1], encfv, memv)
                dq = wk.tile([128, 2 * NSEQ], BF16, tag="dq", name=f"dq{ep}")
                nc.vector.tensor_sub(dq[:, :].rearrange("p (h b i) -> p h b i",
                                                        h=2, i=NF), encfv, qbv)
                nc.scalar.activation(ZT[2][:, :], dq[:, :], AF.Abs)
                dm = wk.tile([128, 2 * NSEQ], BF16, tag="dm", name=f"dm{ep}")
                nc.vector.tensor_sub(dm[:, :].rearrange("p (h b i) -> p h b i",
                                                        h=2, i=NF), encfv, memv)
                nc.scalar.activation(ZT[3][:, :], dm[:, :], AF.Abs)

                p1T = []
                for m in range(2):
                    pp = ps.tile([128, NSEQ], F32, tag="bank", name=f"p1{ep}_{m}")
                    for kt in range(8):
                        nc.tensor.matmul(pp[:, :], g1T[kt][:, m * 128:(m + 1) * 128],
                                         ZT[kt // 2][:, (kt % 2) * NSEQ:
                                                     (kt % 2 + 1) * NSEQ],
                                         start=(kt == 0), stop=(kt == 7))
                    t1 = wk.tile([128, NSEQ], BF16, tag="p1s", bufs=2,
                                 name=f"p1s{ep}_{m}")
                    nc.scalar.activation(t1[:, :], pp[:, :], AF.Tanh,
                                         bias=gb1[:, m:m + 1], scale=1.0)
                    p1T.append(t1)
                pgp = ps.tile([1, NSEQ], F32, tag="bank", name=f"pg{ep}")
                for k in range(2):
                    nc.tensor.matmul(pgp[:, :], g2T[k][:, :], p1T[k][:, :],
                                     start=(k == 0), stop=(k == 1))
                g_row = wk.tile([1, NSEQ], BF16, tag="grow", name=f"grow{ep}")
                nc.scalar.activation(g_row[:, :], pgp[:, :], AF.Sigmoid,
                                     bias=gb2[:, :], scale=1.0)
                pgB = ps.tile([128, 2 * NSEQ], F32, tag="bank", name=f"pgB{ep}")
                nc.tensor.matmul(pgB[:, 0:NSEQ], ones1[:, :], g_row[:, :],
                                 start=True, stop=True)
                nc.tensor.matmul(pgB[:, NSEQ:2 * NSEQ], ones1[:, :], g_row[:, :],
                                 start=True, stop=True)
                gBc = wk.tile([128, 2 * NSEQ], BF16, tag="gBc", name=f"gBc{ep}")
                evict(gBc[:, :], pgB[:, :])
                gBv = gBc[:, :].rearrange("p (h b i) -> p h b i", h=2, i=NF)

                eT = wk.tile([128, 2 * BB], BF16, tag="eTs", name=f"eT{ep}")
                nc.vector.memset(eT[:, :], 0.0)
                e_ap = eT[:, :]
                for i in range(NF):
                    enew = gru2("att", e_ap, BB, girz_av[:, :, :, i],
                                gin_av[:, :, :, i], att_g=gBv[:, :, :, i],
                                name=f"e{ep}_{i}_")
                    e_ap = enew[:, :]
                mnew = gru2("mem", mem_ap, BB, mem_girz, mem_gin,
                            rhs_x=e_ap, xk=(0, 2), name=f"m{ep}_")
                mem_ap = mnew[:, :]

            memF = st.tile([128, 2 * BB], BF16, name="memF")
            nc.vector.tensor_copy(memF[:, :], mem_ap)

            # ================= P4: all-gather mem|enc_q =================
            encrow = wk.tile([BB, 2 * H], F32, name="encrow")
            for ch in range(2):
                pt = ps.tile([BB, 128], BF16, tag="bank", name=f"egm{ch}")
                nc.tensor.transpose(pt[:, :], memF[:, ch * BB:(ch + 1) * BB],
                                    ident[:, :])
                evict(encrow[:, ch * 128:(ch + 1) * 128], pt[:, :])
                pt2 = ps.tile([BB, 128], BF16, tag="bank", name=f"egq{ch}")
                nc.tensor.transpose(pt2[:, :], hqT[:, ch * BB:(ch + 1) * BB],
                                    ident[:, :])
                evict(encrow[:, 256 + ch * 128:256 + (ch + 1) * 128], pt2[:, :])
            nc.sync.dma_start(cc_enc_in[:, :], encrow[:, :])
            nc.gpsimd.collective_compute("AllGather", ALU.bypass,
                                         ins=[cc_enc_in[:, :]],
                                         outs=[cc_enc_out[:, :]], replica_groups=rg)
            enc_all = wk.tile([B, 2 * H], F32, name="enc_all")
            nc.sync.dma_start(enc_all[:, :], cc_enc_out[:, :])

            memA = st.tile([128, 2 * B], BF16, name="memA")
            qA = st.tile([128, 2 * B], BF16, name="qA")
            for ch in range(2):
                pt = ps.tile([128, B], F32, tag="bank", name=f"tmA{ch}")
                nc.tensor.transpose(pt[:, :], enc_all[:, ch * 128:(ch + 1) * 128],
                                    identf[:B, :B])
                evict(memA[:, ch * B:(ch + 1) * B], pt[:, :])
                pt2 = ps.tile([128, B], F32, tag="bank", name=f"tqA{ch}")
                nc.tensor.transpose(pt2[:, :], enc_all[:, 256 + ch * 128:
                                                       256 + (ch + 1) * 128],
                                    identf[:B, :B])
                evict(qA[:, ch * B:(ch + 1) * B], pt2[:, :])

            # ================= P5: decode =================
            iota_i = st.tile([128, CHW], DT.int32)
            nc.gpsimd.iota(iota_i[:, :], pattern=[[1, CHW]], base=0,
                           channel_multiplier=0)
            iota_h = st.tile([128, CHW], F16)
            nc.vector.tensor_copy(iota_h[:, :], iota_i[:, :])

            girz_A = st.tile([128, 4 * B], BF16, name="girz_A")
            gin_A = st.tile([128, 2 * B], BF16, name="gin_A")
            # hoist uses wihT tiles [0:2]; ans q-part lives in tiles [2:4]
            for m in range(6):
                pp = ps.tile([128, B], F32, tag="bank", name=f"dxp{m}")
                for k in range(2):
                    nc.tensor.matmul(pp[:, :],
                                     W["ans_wihT"][2 + k][:, m * 128:(m + 1) * 128],
                                     qA[:, k * B:(k + 1) * B],
                                     start=(k == 0), stop=(k == 1))
                if m < 4:
                    bias = W["ans_brz"][:, m:m + 1]
                    dst = girz_A[:, m * B:(m + 1) * B]
                else:
                    bias = W["ans_bin"][:, m - 4:m - 3]
                    dst = gin_A[:, (m - 4) * B:(m - 3) * B]
                evict(dst, pp[:, :], bias)
            girz_Av = girz_A[:, :].rearrange("p (m t) -> p m t", m=4)
            gin_Av = gin_A[:, :].rearrange("p (m t) -> p m t", m=2)

            hid_ap = memA[:, :]
            last_ap = lastT0[:, :]
            out3 = out_logp.rearrange("(b t) v -> b t v", t=alen)
            W2K = NPAIR * CHW  # 2000 packed cols per partition-half

            Pd = gru2_hmm("ans", hid_ap, B, True, name="a0_")
            for t_step in range(alen):
                h2 = gru2_rest("ans", Pd, hid_ap, B, girz_Av, gin_Av,
                               rhs_x=last_ap, xk=(0, 2), name=f"a{t_step}_")
                hid_ap = h2[:, :]

                lg = wk.tile([128, W2K], BF16, tag="lg", bufs=1,
                             name=f"lg{t_step}")
                sext = wk.tile([128, NPAIR], F32, tag="sext", name=f"sx{t_step}")
                Mt = wk.tile([128, NPAIR], BF16, tag="Mt", name=f"Mt{t_step}")
                It = wk.tile([128, NPAIR], F16, tag="It", name=f"It{t_step}")
                for j in range(NPAIR):
                    pl = ps.tile([128, CHW], F32, tag="bank", name=f"pl{t_step}_{j}")
                    for half in range(2):
                        oap = pl[half * B:(half + 1) * B, :]
                        for k in range(2):
                            nc.tensor.matmul(
                                oap, hid_ap[:, k * B:(k + 1) * B],
                                fcwT[k][:, (2 * j + half) * CHW:
                                         (2 * j + half + 1) * CHW],
                                start=(k == 0), stop=(k == 1),
                                tile_position=(0, 64) if half else None)
                    if fcb_nonzero:
                        nc.vector.tensor_add(pl[:, :], pl[:, :],
                                             fcbp[:, j * CHW:(j + 1) * CHW])
                    evict(lg[:, j * CHW:(j + 1) * CHW], pl[:, :])
                    dump = wk.tile([128, CHW], BF16, tag="dump", bufs=2,
                                   name=f"dump{t_step}_{j}")
                    nc.scalar.activation(dump[:, :], pl[:, :], AF.Exp,
                                         bias=zb[:, :], scale=1.0,
                                         accum_out=sext[:, j:j + 1])
                    # per-pair max on DVE; mask+index on GpSimd (both engines
                    # overlap the remaining pairs' matmuls)
                    nc.vector.tensor_reduce(Mt[:, j:j + 1],
                                            lg[:, j * CHW:(j + 1) * CHW],
                                            axis=mybir.AxisListType.X, op=ALU.max)
                    msk = wk.tile([128, CHW], F16, tag="msk", bufs=2,
                                  name=f"msk{t_step}_{j}")
                    nc.vector.scalar_tensor_tensor(msk[:, :],
                                                   lg[:, j * CHW:(j + 1) * CHW],
                                                   Mt[:, j:j + 1], iota_h[:, :],
                                                   ALU.is_equal, ALU.mult)
                    nc.vector.tensor_reduce(It[:, j:j + 1], msk[:, :],
                                            axis=mybir.AxisListType.X, op=ALU.max)

                c1 = wk.tile([128, 1], BF16, tag="c1", name=f"c1{t_step}")
                nc.vector.tensor_reduce(c1[:, :], Mt[:, :],
                                        axis=mybir.AxisListType.X, op=ALU.max)
                itf = wk.tile([128, NPAIR], F32, tag="itf", name=f"itf{t_step}")
                nc.vector.tensor_copy(itf[:, :], It[:, :])
                itg = wk.tile([128, NPAIR], F32, tag="itg", name=f"itg{t_step}")
                nc.vector.tensor_add(itg[:, :], itf[:, :], offc2[:, :])
                wix = wk.tile([128, NPAIR], F32, tag="wix", name=f"wix{t_step}")
                nc.vector.scalar_tensor_tensor(wix[:, :], Mt[:, :], c1[:, :],
                                               itg[:, :], ALU.is_equal, ALU.mult)
                pack = wk.tile([128, 4], F32, tag="pack", name=f"pack{t_step}")
                nc.vector.tensor_copy(pack[:, 0:1], c1[:, :])
                nc.vector.tensor_reduce(pack[:, 1:2], wix[:, :],
                                        axis=mybir.AxisListType.X, op=ALU.max)
                nc.vector.tensor_reduce(pack[:, 2:3], sext[:, :],
                                        axis=mybir.AxisListType.X, op=ALU.add)
                nc.vector.tensor_copy(pack[:, 3:4], zb[:, :])
                nc.sync.dma_start(cc_top_in[t_step][:, :], pack[:, :])
                nc.gpsimd.collective_compute("AllGather", ALU.bypass,
                                             ins=[cc_top_in[t_step][:, :]],
                                             outs=[cc_top_out[t_step][:, :]],
                                             replica_groups=rg)
                # next step's h-side matmuls run on the PE during the AG
                if t_step < alen - 1:
                    Pd = gru2_hmm("ans", hid_ap, B, True, name=f"a{t_step + 1}_")
                # readback: row = core*128 + half*64 + batch
                topall = wk.tile([B, 16 * 4], F32, tag="topall",
                                 name=f"topall{t_step}")
                nc.sync.dma_start(
                    topall[:, :].rearrange("b (c v) -> b c v", v=4),
                    cc_top_out[t_step].rearrange("(c h b) v -> b (c h) v",
                                                 h=2, b=B),
                )
                t3 = topall[:, :].rearrange("b (c v) -> b c v", v=4)
                gv = wk.tile([B, 1], F32, tag="gv", name=f"gv{t_step}")
                nc.vector.tensor_reduce(gv[:, :], t3[:, :, 0],
                                        axis=mybir.AxisListType.X, op=ALU.max)
                wtokf = wk.tile([B, 16], F32, tag="wtokf", name=f"wtokf{t_step}")
                nc.vector.scalar_tensor_tensor(wtokf[:, :], t3[:, :, 0], gv[:, :],
                                               t3[:, :, 1], ALU.is_equal, ALU.mult)
                wtok = wk.tile([B, 1], F32, tag="wtok", name=f"wtok{t_step}")
                nc.vector.tensor_reduce(wtok[:, :], wtokf[:, :],
                                        axis=mybir.AxisListType.X, op=ALU.max)
                stot = wk.tile([B, 1], F32, tag="stot", name=f"stot{t_step}")
                nc.vector.tensor_reduce(stot[:, :], t3[:, :, 2],
                                        axis=mybir.AxisListType.X, op=ALU.add)
                nlz = wk.tile([128, 1], F32, tag="nlz", name=f"nlz{t_step}")
                nc.scalar.activation(nlz[0:B, :], stot[:, :], AF.Ln,
                                     bias=zb[0:B, :], scale=1.0)
                nc.vector.tensor_scalar_mul(nlz[0:B, :], nlz[0:B, :], -1.0)
                nc.sync.dma_start(nlz[B:2 * B, :], nlz[0:B, :])
                for j in range(NPAIR):
                    og = wk.tile([128, CHW], F32, tag="og", bufs=2,
                                 name=f"og{t_step}_{j}")
                    if j % 2 == 0:
                        nc.scalar.activation(og[:, :], lg[:, j * CHW:(j + 1) * CHW],
                                             AF.Identity, bias=nlz[:, :], scale=1.0)
                    else:
                        nc.vector.tensor_scalar(og[:, :], lg[:, j * CHW:(j + 1) * CHW],
                                                nlz[:, :], None, ALU.add)
                    nc.sync.dma_start(out3[:, t_step, j * CHW:(j + 1) * CHW],
                                      og[0:B, :])
                    nc.sync.dma_start(
                        out3[:, t_step, 2000 + j * CHW:2000 + (j + 1) * CHW],
                        og[B:2 * B, :])
                if t_step == alen - 1:
                    continue
                wtoki = wk.tile([B, 1], DT.int32, tag="wtoki", name=f"wtoki{t_step}")
                nc.vector.tensor_copy(wtoki[:, :], wtok[:, :])
                lemb = wk.tile([B, E], BF16, tag="lemb", name=f"lemb{t_step}")
                nc.gpsimd.indirect_dma_start(
                    out=lemb[:, :], out_offset=None, in_=io["embed"][:, :],
                    in_offset=bass.IndirectOffsetOnAxis(ap=wtoki[:, :1], axis=0),
                )
                newlast = wk.tile([128, 2 * B], BF16, tag="lastT", bufs=2,
                                  name=f"lastT{t_step}")
                for ch in range(2):
                    pt = ps.tile([128, B], BF16, tag="bank", name=f"lt{t_step}_{ch}")
                    nc.tensor.transpose(pt[:, :], lemb[:, ch * 128:(ch + 1) * 128],
                                        ident[:B, :B])
                    evict(newlast[:, ch * B:(ch + 1) * B], pt[:, :])
                last_ap = newlast[:, :]

    nc.finalize()
    return nc


def prep_inputs(inputs):
    """Host-side shard/pack. Returns in_maps list for the 8 cores."""
    f32 = np.float32
    bf16 = ml_dtypes.bfloat16
    emb = np.ascontiguousarray(np.asarray(inputs["embed_w"], f32).astype(bf16))
    packs = {}
    for g in GK:
        wih = np.asarray(inputs[f"{g}_wih"], dtype=f32)
        whh = np.asarray(inputs[f"{g}_whh"], dtype=f32)
        bih = np.asarray(inputs[f"{g}_bih"], dtype=f32)
        bhh = np.asarray(inputs[f"{g}_bhh"], dtype=f32)
        brz = bih[:512] + bhh[:512]
        packs[f"{g}_wihT"] = np.ascontiguousarray(wih.T.astype(bf16))
        packs[f"{g}_whhT"] = np.ascontiguousarray(whh.T.astype(bf16))
        packs[f"{g}_brz"] = np.ascontiguousarray(brz.reshape(4, 128).T)
        packs[f"{g}_bin"] = np.ascontiguousarray(bih[512:768].reshape(2, 128).T)
        packs[f"{g}_bhn"] = np.ascontiguousarray(bhh[512:768].reshape(2, 128).T)
        packs[f"{g}_bhnr"] = np.ascontiguousarray(
            bhh[512:768].reshape(1, 256).astype(bf16))
    packs["g1T"] = np.ascontiguousarray(np.asarray(inputs["gate_w1"], f32).T.astype(bf16))
    packs["g2T"] = np.ascontiguousarray(np.asarray(inputs["gate_w2"], f32).T.astype(bf16))
    packs["gb1"] = np.ascontiguousarray(np.asarray(inputs["gate_b1"], f32).reshape(2, 128).T)
    packs["gb2"] = np.asarray(inputs["gate_b2"], f32).reshape(1, 1)
    fcwT = np.ascontiguousarray(np.asarray(inputs["fc_w"], f32).T.astype(bf16))
    fcb = np.asarray(inputs["fc_b"], f32)
    fcb_nonzero = bool(np.any(fcb != 0))
    last0T = np.ascontiguousarray(
        np.tile(np.asarray(inputs["embed_w"], f32)[SEQBEGIN][:, None],
                (1, B)).astype(bf16))
    allfacts = np.asarray(inputs["allfacts"], np.int32)
    questions = np.asarray(inputs["questions"], np.int32)

    in_maps = []
    for j in range(NCORE):
        m = dict(packs)
        m["embed"] = emb
        # permute the vocab shard so packed col c on partition-half h maps to
        # token j*VS + h*2000 + c: new[:, jj*1000 + h*500 + r] = old h*2000+jj*500+r
        shard = fcwT[:, j * VS:(j + 1) * VS]
        sh3 = shard.reshape(E, 2, NPAIR, CHW)          # (h, jj, r)
        m["fcwT"] = np.ascontiguousarray(
            sh3.transpose(0, 2, 1, 3).reshape(E, VS))  # (jj, h, r)
        oc = np.zeros((128, NPAIR), f32)
        for jj in range(NPAIR):
            oc[0:B, jj] = j * VS + jj * CHW
            oc[B:128, jj] = j * VS + 2000 + jj * CHW
        m["offc2"] = oc
        if fcb_nonzero:
            fcbp = np.zeros((128, NPAIR * CHW), f32)
            sh = fcb[j * VS:(j + 1) * VS]
            for jj in range(NPAIR):
                fcbp[0:B, jj * CHW:(jj + 1) * CHW] = sh[jj * CHW:(jj + 1) * CHW]
                fcbp[B:128, jj * CHW:(jj + 1) * CHW] = \
                    sh[2000 + jj * CHW:2000 + (jj + 1) * CHW]
            m["fcbp"] = fcbp
        m["last0T"] = last0T
        m["facts_idx"] = np.ascontiguousarray(
            allfacts[j * BB:(j + 1) * BB].reshape(NSEQ, FL).T.reshape(-1, 1))
        m["q_idx"] = np.ascontiguousarray(
            questions[j * BB:(j + 1) * BB].reshape(BB, QL).T.reshape(-1, 1))
        in_maps.append(m)
    return in_maps, fcb_nonzero


_CACHE = {}


def kernel(**inputs):
    alen = int(inputs["alen"])
    in_maps, fcb_nonzero = prep_inputs(inputs)
    key = (alen, fcb_nonzero)
    if key not in _CACHE:
        _CACHE[key] = build_nc(alen, fcb_nonzero)
    nc = _CACHE[key]
    res = run_bass_kernel_spmd(nc, in_maps, core_ids=list(range(NCORE)))
    out = np.concatenate([res.results[j]["out_logp"] for j in range(NCORE)], axis=1)
    return out.astype(np.float32)


# revision 28
# speedup vs baseline: 1.0433x; 1.0319x over previous
"""DMN forward on 8 Trainium2 NeuronCores (Bass/Tile), v3 (bf16).

Sharding: batch rows 8/core for fact+question encoding and episodic memory;
decode GRU replicated on all cores, fc/log-softmax vocab-sharded 4000
columns/core. One tiny AllGather per decode step carries per-half
(max, argmax-token, sumexp); a dummy warmup AllGather early in the kernel
absorbs the first-collective latency while the fact GRU runs.

v3 structure:
- all matmuls bf16 (host-validated: relnorm ~1e-3 vs fp32 ref, gate 2e-2)
- GRU state as [128, 2n] combined-halves tiles; r and z share one PSUM tile
  and one Sigmoid (4n <= 512 everywhere after splitting facts into groups)
- the fact recurrence runs as 2 independent 80-sequence chains, interleaved,
  so each chain's elementwise latency hides under the other's matmuls
- episodic attention blend fused into the GRU tail:
  e' = e + g*(1-z)*(n - e)  (exact rewrite of blend(GRU))
- fc weights host-permuted so the packed [128, 2000] logit tile has a linear
  token map: token = halfv[p] + col; argmax is 3 full-width 16-bit DVE ops
"""

import os
import numpy as np
import ml_dtypes

import concourse.bass as bass
import concourse.bacc as bacc
import concourse.mybir as mybir
from concourse.tile import TileContext
from concourse.bass_utils import run_bass_kernel_spmd
from concourse.masks import make_identity

AF = mybir.ActivationFunctionType
ALU = mybir.AluOpType
DT = mybir.dt
BF16 = DT.bfloat16
F32 = DT.float32
F16 = DT.float16

V, E, H = 32000, 256, 256
B, NF, FL, QL = 64, 20, 32, 16
N_EPISODE = 3
SEQBEGIN = 1
NCORE = 8
BB = B // NCORE            # 8 batch rows per core
NSEQ = BB * NF             # 160 fact seqs per core
NTOK = NSEQ * FL           # 5120 fact tokens per core
VS = V // NCORE            # 4000 vocab shard
CHW = 500                  # fc chunk width (one PSUM half)
NPAIR = 4                  # 4 pairs x (2x500) = 4000
GF = 2                     # fact-recurrence groups
GS = NSEQ // GF            # 80 seqs per group

GK = {"ig": E, "qg": E, "att": H, "mem": H, "ans": 2 * H}


def build_nc(alen, fcb_nonzero):
    nc = bacc.Bacc("TRN2", num_devices=NCORE)

    def dram_in(name, shape, dtype=F32):
        return nc.dram_tensor(name, list(shape), dtype, kind="ExternalInput")

    io = {}
    io["facts_idx"] = dram_in("facts_idx", [NTOK, 1], DT.int32)
    io["q_idx"] = dram_in("q_idx", [BB * QL, 1], DT.int32)
    io["embed"] = dram_in("embed", [V, E], BF16)
    io["fcwT"] = dram_in("fcwT", [E, VS], BF16)
    io["last0T"] = dram_in("last0T", [E, B], BF16)
    io["halfv"] = dram_in("halfv", [128, 1])
    if fcb_nonzero:
        io["fcbp"] = dram_in("fcbp", [128, NPAIR * CHW])
    for g, kin in GK.items():
        io[f"{g}_wihT"] = dram_in(f"{g}_wihT", [kin, 3 * H], BF16)
        io[f"{g}_whhT"] = dram_in(f"{g}_whhT", [H, 3 * H], BF16)
        io[f"{g}_brz"] = dram_in(f"{g}_brz", [128, 4])
        io[f"{g}_bin"] = dram_in(f"{g}_bin", [128, 2])
        io[f"{g}_bhn"] = dram_in(f"{g}_bhn", [128, 2])
    io["g1T"] = dram_in("g1T", [4 * H, H], BF16)
    io["g2T"] = dram_in("g2T", [H, 1], BF16)
    io["gb1"] = dram_in("gb1", [128, 2])
    io["gb2"] = dram_in("gb2", [1, 1])

    out_logp = nc.dram_tensor("out_logp", [B * alen, VS], F32, kind="ExternalOutput")

    cc_warm_in = nc.dram_tensor("cc_warm_in", [8, 4], F32, kind="Internal")
    cc_warm_out = nc.dram_tensor("cc_warm_out", [NCORE * 8, 4], F32, kind="Internal",
                                 addr_space="Shared")
    cc_enc_in = nc.dram_tensor("cc_enc_in", [BB, 2 * H], F32, kind="Internal")
    cc_enc_out = nc.dram_tensor("cc_enc_out", [B, 2 * H], F32, kind="Internal",
                                addr_space="Shared")
    cc_top_in = [nc.dram_tensor(f"cc_top_in{t}", [128, 4], F32, kind="Internal")
                 for t in range(alen)]
    cc_top_out = [nc.dram_tensor(f"cc_top_out{t}", [NCORE * 128, 4], F32,
                                 kind="Internal", addr_space="Shared")
                  for t in range(alen)]
    rg = [list(range(NCORE))]

    with TileContext(nc) as tc:
        with tc.tile_pool(name="state", bufs=1) as st, \
             tc.tile_pool(name="work", bufs=2) as wk, \
             tc.tile_pool(name="ps", bufs=8, space="PSUM") as ps:

            ident = st.tile([128, 128], BF16)
            make_identity(nc, ident[:, :])
            identf = st.tile([128, 128], F32)
            make_identity(nc, identf[:, :])
            ones1 = st.tile([1, 128], BF16)
            nc.vector.memset(ones1[:, :], 1.0)
            zb = st.tile([128, 1], F32)
            nc.vector.memset(zb[:, :], 0.0)

            # ---- load all weights up front (DMA overlaps everything) ----
            W = {}
            for g, kin in GK.items():
                W[f"{g}_wihT"] = []
                for k in range(kin // 128):
                    t = st.tile([128, 3 * H], BF16, name=f"{g}wih{k}")
                    nc.sync.dma_start(t[:, :], io[f"{g}_wihT"][k * 128:(k + 1) * 128, :])
                    W[f"{g}_wihT"].append(t)
                W[f"{g}_whhT"] = []
                for k in range(2):
                    t = st.tile([128, 3 * H], BF16, name=f"{g}whh{k}")
                    nc.sync.dma_start(t[:, :], io[f"{g}_whhT"][k * 128:(k + 1) * 128, :])
                    W[f"{g}_whhT"].append(t)
                for bn, w in (("brz", 4), ("bin", 2), ("bhn", 2)):
                    t = st.tile([128, w], F32, name=f"{g}{bn}")
                    nc.sync.dma_start(t[:, :], io[f"{g}_{bn}"][:, :])
                    W[f"{g}_{bn}"] = t
            g1T = []
            for k in range(8):
                t = st.tile([128, H], BF16, name=f"g1T{k}")
                nc.sync.dma_start(t[:, :], io["g1T"][k * 128:(k + 1) * 128, :])
                g1T.append(t)
            g2T = []
            for k in range(2):
                t = st.tile([128, 1], BF16, name=f"g2T{k}")
                nc.sync.dma_start(t[:, :], io["g2T"][k * 128:(k + 1) * 128, :])
                g2T.append(t)
            gb1 = st.tile([128, 2], F32)
            nc.sync.dma_start(gb1[:, :], io["gb1"][:, :])
            gb2 = st.tile([1, 1], F32)
            nc.sync.dma_start(gb2[:, :], io["gb2"][:, :])
            fcwT = []
            for k in range(2):
                t = st.tile([128, VS], BF16, name=f"fcwT{k}")
                nc.sync.dma_start(t[:, :], io["fcwT"][k * 128:(k + 1) * 128, :])
                fcwT.append(t)
            lastT0 = st.tile([128, 2 * B], BF16, name="lastT0")
            for k in range(2):
                nc.sync.dma_start(lastT0[:, k * B:(k + 1) * B],
                                  io["last0T"][k * 128:(k + 1) * 128, :])
            halfv = st.tile([128, 1], F32)
            nc.sync.dma_start(halfv[:, :], io["halfv"][:, :])
            if fcb_nonzero:
                fcbp = st.tile([128, NPAIR * CHW], F32)
                nc.sync.dma_start(fcbp[:, :], io["fcbp"][:, :])

            evict_rr = [0]

            def evict(dst_ap, src_ap, bias=None):
                """PSUM -> SBUF eviction, alternating DVE/ACT, optional
                per-partition bias add (bias: [128,1] AP)."""
                if bias is None:
                    if evict_rr[0] % 2 == 0:
                        nc.vector.tensor_copy(dst_ap, src_ap)
                    else:
                        nc.scalar.activation(dst_ap, src_ap, AF.Copy)
                else:
                    if evict_rr[0] % 2 == 0:
                        nc.vector.tensor_scalar(dst_ap, src_ap, bias, None, ALU.add)
                    else:
                        nc.scalar.activation(dst_ap, src_ap, AF.Identity,
                                             bias=bias, scale=1.0)
                evict_rr[0] += 1

            def v3(ap2d, n):
                return ap2d.rearrange("p (c n) -> p c n", c=2)

            def v4(ap2d, n):
                return ap2d.rearrange("p (c n) -> p c n", c=4)

            # ================= GRU cell, combined-halves, fused r+z ==========
            def gru2_hmm(g, h_ap, n, has_x, name=""):
                """h-side matmuls of a GRU step into fresh PSUM tiles: prz
                [128, 4n] (r,z gate quarters) + pnh [128, 2n] (+pni for x's
                n-gate when has_x). With has_x the r/z groups stay open."""
                hw = W[f"{g}_whhT"]
                prz = ps.tile([128, 4 * n], F32, tag="bank", name=f"{name}prz")
                pnh = ps.tile([128, 2 * n], F32, tag="bank", name=f"{name}pnh")
                pni = ps.tile([128, 2 * n], F32, tag="bank", name=f"{name}pni") \
                    if has_x else None
                for m in range(6):
                    dd = prz[:, m * n:(m + 1) * n] if m < 4 else \
                        pnh[:, (m - 4) * n:(m - 3) * n]
                    last_h = (not has_x) or (m >= 4)
                    for k in range(2):
                        nc.tensor.matmul(dd, hw[k][:, m * 128:(m + 1) * 128],
                                         h_ap[:, k * n:(k + 1) * n],
                                         start=(k == 0),
                                         stop=(k == 1 and last_h))
                return (prz, pnh, pni)

            def gru2_rest(g, P, h_ap, n, gi_rz, gi_n, rhs_x=None, xk=(0, 2),
                          att_g=None, name=""):
                """Finish a GRU step: x-side matmuls (if any) + elementwise.
                gi_rz: [128, 4, n] AP (x-proj r/z quarters, biases folded).
                gi_n: [128, 2, n] AP. att_g: optional [128, 2, n] gate AP g_i;
                when given, returns the fused episode update
                e + g*(1-z)*(tanh(u) - e) instead of the plain GRU output."""
                prz, pnh, pni = P
                if rhs_x is not None:
                    xw = W[f"{g}_wihT"][xk[0]:xk[1]]
                    for m in range(6):
                        dd = prz[:, m * n:(m + 1) * n] if m < 4 else \
                            pni[:, (m - 4) * n:(m - 3) * n]
                        for k in range(2):
                            nc.tensor.matmul(dd, xw[k][:, m * 128:(m + 1) * 128],
                                             rhs_x[:, k * n:(k + 1) * n],
                                             start=(m >= 4 and k == 0),
                                             stop=(k == 1))

                arz = wk.tile([128, 4 * n], BF16, tag=f"arz{n}", name=f"{name}arz")
                nc.vector.tensor_add(v4(arz[:, :], n), v4(prz[:, :], n), gi_rz)
                rz = wk.tile([128, 4 * n], BF16, tag=f"rz{n}", name=f"{name}rz")
                nc.scalar.activation(rz[:, :], arz[:, :], AF.Sigmoid, bias=zb[:, :],
                                     scale=1.0)
                r = rz[:, 0:2 * n]
                z = rz[:, 2 * n:4 * n]
                bhn_b = W[f"{g}_bhn"][:, :].to_broadcast([128, 2, n])
                t1 = wk.tile([128, 2 * n], BF16, tag=f"t1{n}", name=f"{name}t1")
                nc.vector.tensor_add(v3(t1[:, :], n), v3(pnh[:, :], n), bhn_b)
                y = wk.tile([128, 2 * n], BF16, tag=f"y{n}", name=f"{name}y")
                nc.vector.tensor_mul(y[:, :], t1[:, :], r)
                u = wk.tile([128, 2 * n], BF16, tag=f"u{n}", name=f"{name}u")
                if pni is not None:
                    t2 = wk.tile([128, 2 * n], BF16, tag=f"t2{n}", name=f"{name}t2")
                    nc.vector.tensor_add(v3(t2[:, :], n), v3(pni[:, :], n), gi_n)
                    nc.vector.tensor_add(u[:, :], y[:, :], t2[:, :])
                else:
                    nc.vector.tensor_add(v3(u[:, :], n), v3(y[:, :], n), gi_n)
                omz = wk.tile([128, 2 * n], BF16, tag=f"omz{n}", name=f"{name}omz")
                nc.vector.tensor_scalar(omz[:, :], z, -1.0, 1.0, ALU.mult, ALU.add)
                nn = wk.tile([128, 2 * n], BF16, tag=f"nn{n}", name=f"{name}nn")
                nc.scalar.activation(nn[:, :], u[:, :], AF.Tanh, bias=zb[:, :],
                                     scale=1.0)
                h2 = wk.tile([128, 2 * n], BF16, tag=f"h2{n}", bufs=4,
                             name=f"{name}h2")
                if att_g is None:
                    zh = wk.tile([128, 2 * n], BF16, tag=f"zh{n}", name=f"{name}zh")
                    nc.vector.tensor_mul(zh[:, :], z, h_ap)
                    e1 = wk.tile([128, 2 * n], BF16, tag=f"e1{n}", name=f"{name}e1")
                    nc.vector.tensor_mul(e1[:, :], nn[:, :], omz[:, :])
                    nc.vector.tensor_add(h2[:, :], e1[:, :], zh[:, :])
                else:
                    c2 = wk.tile([128, 2 * n], BF16, tag=f"c2{n}", name=f"{name}c2")
                    nc.vector.tensor_mul(v3(c2[:, :], n), v3(omz[:, :], n), att_g)
                    dd2 = wk.tile([128, 2 * n], BF16, tag=f"dd{n}", name=f"{name}dd")
                    nc.vector.tensor_sub(dd2[:, :], nn[:, :], h_ap)
                    p2 = wk.tile([128, 2 * n], BF16, tag=f"p2{n}", name=f"{name}p2")
                    nc.vector.tensor_mul(p2[:, :], c2[:, :], dd2[:, :])
                    nc.vector.tensor_add(h2[:, :], h_ap, p2[:, :])
                return h2

            def gru2(g, h_ap, n, gi_rz, gi_n, rhs_x=None, xk=(0, 2), att_g=None,
                     name=""):
                P = gru2_hmm(g, h_ap, n, rhs_x is not None, name=name)
                return gru2_rest(g, P, h_ap, n, gi_rz, gi_n, rhs_x=rhs_x, xk=xk,
                                 att_g=att_g, name=name)

            # helper: hoist an x-projection (all biases folded) into
            # girz [128, 4*N] + gin [128, 2*N] tiles, N tokens from rhs
            def hoist(g, rhs_fn, N, girz, gin, c0, c1, cw, tag):
                for c in range(c0, c1):
                    for m in range(6):
                        pp = ps.tile([128, cw], F32, tag="bank",
                                     name=f"{tag}xp{m}_{c}")
                        for k in range(2):
                            nc.tensor.matmul(pp[:, :],
                                             W[f"{g}_wihT"][k][:, m * 128:(m + 1) * 128],
                                             rhs_fn(k, c), start=(k == 0),
                                             stop=(k == 1))
                        if m < 4:
                            bias = W[f"{g}_brz"][:, m:m + 1]
                            dst = girz[:, m * N + c * cw:m * N + (c + 1) * cw]
                        else:
                            bias = W[f"{g}_bin"][:, m - 4:m - 3]
                            dst = gin[:, (m - 4) * N + c * cw:(m - 4) * N + (c + 1) * cw]
                        evict(dst, pp[:, :], bias)

            # ============ P1: gather facts + transpose + hoist x-proj ============
            fp_cm = tc.tile_pool(name="fpool", bufs=1)
            fp = fp_cm.__enter__()
            fidx = st.tile([128, NTOK // 128], DT.int32, name="fidx")
            nc.sync.dma_start(fidx[:, :], io["facts_idx"].rearrange(
                "(b a) o -> a (b o)", a=128))
            qidx = st.tile([128, 1], DT.int32, name="qidx")
            nc.sync.dma_start(qidx[:, :], io["q_idx"][:, :])

            XT = [fp.tile([128, NTOK], BF16, name=f"XT{k}") for k in range(2)]
            girz_f = fp.tile([128, 4 * NTOK], BF16, name="girz_f")
            gin_f = fp.tile([128, 2 * NTOK], BF16, name="gin_f")

            for i in range(NTOK // 128):
                gt = wk.tile([128, E], BF16, tag="fgat", bufs=4, name=f"fg{i}")
                nc.gpsimd.indirect_dma_start(
                    out=gt[:, :], out_offset=None, in_=io["embed"][:, :],
                    in_offset=bass.IndirectOffsetOnAxis(ap=fidx[:, i:i + 1], axis=0),
                )
                for ch in range(2):
                    pt = ps.tile([128, 128], BF16, tag="bank", name=f"ftp{i}_{ch}")
                    nc.tensor.transpose(pt[:, :], gt[:, ch * 128:(ch + 1) * 128],
                                        ident[:, :])
                    evict(XT[ch][:, i * 128:(i + 1) * 128], pt[:, :])
                if i % 4 == 3:
                    c = i // 4
                    hoist("ig", lambda k, c: XT[k][:, c * 512:(c + 1) * 512],
                          NTOK, girz_f[:, :], gin_f[:, :], c, c + 1, 512, "f")

            # q gather + hoist (BB*QL = 128 tokens)
            qg_t = wk.tile([128, E], BF16, tag="fgat", bufs=4, name="qgat")
            nc.gpsimd.indirect_dma_start(
                out=qg_t[:, :], out_offset=None, in_=io["embed"][:, :],
                in_offset=bass.IndirectOffsetOnAxis(ap=qidx[:, :1], axis=0),
            )
            # warmup collective: pays the first-collective setup cost on the
            # (separate) collective hardware while the fact GRU runs.
            nc.gpsimd.collective_compute("AllGather", ALU.bypass,
                                         ins=[cc_warm_in[:, :]],
                                         outs=[cc_warm_out[:, :]],
                                         replica_groups=rg)
            NQ = BB * QL
            XQT = fp.tile([128, 2 * NQ], BF16, name="XQT")
            for ch in range(2):
                pt = ps.tile([128, 128], BF16, tag="bank", name=f"qtp{ch}")
                nc.tensor.transpose(pt[:, :], qg_t[:, ch * 128:(ch + 1) * 128],
                                    ident[:, :])
                evict(XQT[:, ch * NQ:(ch + 1) * NQ], pt[:, :])
            girz_q = fp.tile([128, 4 * NQ], BF16, name="girz_q")
            gin_q = fp.tile([128, 2 * NQ], BF16, name="gin_q")
            hoist("qg", lambda k, c: XQT[:, k * NQ:(k + 1) * NQ], NQ,
                  girz_q[:, :], gin_q[:, :], 0, 1, NQ, "q")

            # ========= P2: fact (2 groups) + question recurrences =========
            girz_fv = girz_f[:, :].rearrange("p (m t) -> p m t", m=4)
            gin_fv = gin_f[:, :].rearrange("p (m t) -> p m t", m=2)
            girz_qv = girz_q[:, :].rearrange("p (m t) -> p m t", m=4)
            gin_qv = gin_q[:, :].rearrange("p (m t) -> p m t", m=2)

            hfs = []
            for gidx in range(GF):
                t = wk.tile([128, 2 * GS], BF16, tag="hfs", name=f"hf_init{gidx}")
                nc.vector.memset(t[:, :], 0.0)
                hfs.append(t[:, :])
            hq = wk.tile([128, 2 * BB], BF16, tag="hqs", name="hq_init")
            nc.vector.memset(hq[:, :], 0.0)
            hq_ap = hq[:, :]

            for step in range(FL):
                Ps = [gru2_hmm("ig", hfs[gidx], GS, False, name=f"f{step}g{gidx}_")
                      for gidx in range(GF)]
                for gidx in range(GF):
                    base = step * NSEQ + gidx * GS
                    h2 = gru2_rest("ig", Ps[gidx], hfs[gidx], GS,
                                   girz_fv[:, :, base:base + GS],
                                   gin_fv[:, :, base:base + GS],
                                   name=f"f{step}g{gidx}_")
                    hfs[gidx] = h2[:, :]
                if step % 2 == 0 and step // 2 < QL:
                    qs = step // 2
                    q2 = gru2("qg", hq_ap, BB,
                              girz_qv[:, :, qs * BB:(qs + 1) * BB],
                              gin_qv[:, :, qs * BB:(qs + 1) * BB],
                              name=f"q{qs}_")
                    hq_ap = q2[:, :]

            encfT = st.tile([128, 2 * NSEQ], BF16, name="encfT")
            encf_v = encfT[:, :].rearrange("p (h t) -> p h t", h=2)
            for gidx in range(GF):
                nc.vector.tensor_copy(encf_v[:, :, gidx * GS:(gidx + 1) * GS],
                                      v3(hfs[gidx], GS))
            hqT = st.tile([128, 2 * BB], BF16, name="hqT")
            nc.vector.tensor_copy(hqT[:, :], hq_ap)
            fp_cm.__exit__(None, None, None)

            # ================= P3: episodes =================
            girz_a = st.tile([128, 4 * NSEQ], BF16, name="girz_a")
            gin_a = st.tile([128, 2 * NSEQ], BF16, name="gin_a")
            hoist("att", lambda k, c: encfT[:, k * NSEQ:(k + 1) * NSEQ], NSEQ,
                  girz_a[:, :], gin_a[:, :], 0, 1, NSEQ, "a")
            girz_av = girz_a[:, :].rearrange("p (m b i) -> p m b i", m=4, i=NF)
            gin_av = gin_a[:, :].rearrange("p (m b i) -> p m b i", m=2, i=NF)

            mem_girz = W["mem_brz"][:, 0:4].to_broadcast([128, 4, BB])
            mem_gin = W["mem_bin"][:, 0:2].to_broadcast([128, 2, BB])

            memT = st.tile([128, 2 * BB], BF16, name="memT")
            nc.vector.tensor_copy(memT[:, :], hqT[:, :])
            mem_ap = memT[:, :]

            encfv = encfT[:, :].rearrange("p (h b i) -> p h b i", h=2, i=NF)
            qbv = hqT[:, :].rearrange("p (h b) -> p h b", h=2).to_broadcast(
                [128, 2, BB, NF])

            for ep in range(N_EPISODE):
                memv = mem_ap.rearrange("p (h b) -> p h b", h=2).to_broadcast(
                    [128, 2, BB, NF])
                ZT = [wk.tile([128, 2 * NSEQ], BF16, tag=f"zt{x}", bufs=2,
                              name=f"ZT{ep}_{x}") for x in range(4)]
                zv = [t[:, :].rearrange("p (h b i) -> p h b i", h=2, i=NF)
                      for t in ZT]
                nc.vector.tensor_mul(zv[0], encfv, qbv)
                nc.vector.tensor_mul(zv[1], encfv, memv)
                dq = wk.tile([128, 2 * NSEQ], BF16, tag="dq", name=f"dq{ep}")
                nc.vector.tensor_sub(dq[:, :].rearrange("p (h b i) -> p h b i",
                                                        h=2, i=NF), encfv, qbv)
                nc.scalar.activation(ZT[2][:, :], dq[:, :], AF.Abs)
                dm = wk.tile([128, 2 * NSEQ], BF16, tag="dm", name=f"dm{ep}")
                nc.vector.tensor_sub(dm[:, :].rearrange("p (h b i) -> p h b i",
                                                        h=2, i=NF), encfv, memv)
                nc.scalar.activation(ZT[3][:, :], dm[:, :], AF.Abs)

                p1T = []
                for m in range(2):
                    pp = ps.tile([128, NSEQ], F32, tag="bank", name=f"p1{ep}_{m}")
                    for kt in range(8):
                        nc.tensor.matmul(pp[:, :], g1T[kt][:, m * 128:(m + 1) * 128],
                                         ZT[kt // 2][:, (kt % 2) * NSEQ:
                                                     (kt % 2 + 1) * NSEQ],
                                         start=(kt == 0), stop=(kt == 7))
                    t1 = wk.tile([128, NSEQ], BF16, tag="p1s", bufs=2,
                                 name=f"p1s{ep}_{m}")
                    nc.scalar.activation(t1[:, :], pp[:, :], AF.Tanh,
                                         bias=gb1[:, m:m + 1], scale=1.0)
                    p1T.append(t1)
                pgp = ps.tile([1, NSEQ], F32, tag="bank", name=f"pg{ep}")
                for k in range(2):
                    nc.tensor.matmul(pgp[:, :], g2T[k][:, :], p1T[k][:, :],
                                     start=(k == 0), stop=(k == 1))
                g_row = wk.tile([1, NSEQ], BF16, tag="grow", name=f"grow{ep}")
                nc.scalar.activation(g_row[:, :], pgp[:, :], AF.Sigmoid,
                                     bias=gb2[:, :], scale=1.0)
                pgB = ps.tile([128, 2 * NSEQ], F32, tag="bank", name=f"pgB{ep}")
                nc.tensor.matmul(pgB[:, 0:NSEQ], ones1[:, :], g_row[:, :],
                                 start=True, stop=True)
                nc.tensor.matmul(pgB[:, NSEQ:2 * NSEQ], ones1[:, :], g_row[:, :],
                                 start=True, stop=True)
                gBc = wk.tile([128, 2 * NSEQ], BF16, tag="gBc", name=f"gBc{ep}")
                evict(gBc[:, :], pgB[:, :])
                gBv = gBc[:, :].rearrange("p (h b i) -> p h b i", h=2, i=NF)

                eT = wk.tile([128, 2 * BB], BF16, tag="eTs", name=f"eT{ep}")
                nc.vector.memset(eT[:, :], 0.0)
                e_ap = eT[:, :]
                for i in range(NF):
                    enew = gru2("att", e_ap, BB, girz_av[:, :, :, i],
                                gin_av[:, :, :, i], att_g=gBv[:, :, :, i],
                                name=f"e{ep}_{i}_")
                    e_ap = enew[:, :]
                mnew = gru2("mem", mem_ap, BB, mem_girz, mem_gin,
                            rhs_x=e_ap, xk=(0, 2), name=f"m{ep}_")
                mem_ap = mnew[:, :]

            memF = st.tile([128, 2 * BB], BF16, name="memF")
            nc.vector.tensor_copy(memF[:, :], mem_ap)

            # ================= P4: all-gather mem|enc_q =================
            encrow = wk.tile([BB, 2 * H], F32, name="encrow")
            for ch in range(2):
                pt = ps.tile([BB, 128], BF16, tag="bank", name=f"egm{ch}")
                nc.tensor.transpose(pt[:, :], memF[:, ch * BB:(ch + 1) * BB],
                                    ident[:, :])
                evict(encrow[:, ch * 128:(ch + 1) * 128], pt[:, :])
                pt2 = ps.tile([BB, 128], BF16, tag="bank", name=f"egq{ch}")
                nc.tensor.transpose(pt2[:, :], hqT[:, ch * BB:(ch + 1) * BB],
                                    ident[:, :])
                evict(encrow[:, 256 + ch * 128:256 + (ch + 1) * 128], pt2[:, :])
            nc.sync.dma_start(cc_enc_in[:, :], encrow[:, :])
            nc.gpsimd.collective_compute("AllGather", ALU.bypass,
                                         ins=[cc_enc_in[:, :]],
                                         outs=[cc_enc_out[:, :]], replica_groups=rg)
            enc_all = wk.tile([B, 2 * H], F32, name="enc_all")
            nc.sync.dma_start(enc_all[:, :], cc_enc_out[:, :])

            memA = st.tile([128, 2 * B], BF16, name="memA")
            qA = st.tile([128, 2 * B], BF16, name="qA")
            for ch in range(2):
                pt = ps.tile([128, B], F32, tag="bank", name=f"tmA{ch}")
                nc.tensor.transpose(pt[:, :], enc_all[:, ch * 128:(ch + 1) * 128],
                                    identf[:B, :B])
                evict(memA[:, ch * B:(ch + 1) * B], pt[:, :])
                pt2 = ps.tile([128, B], F32, tag="bank", name=f"tqA{ch}")
                nc.tensor.transpose(pt2[:, :], enc_all[:, 256 + ch * 128:
                                                       256 + (ch + 1) * 128],
                                    identf[:B, :B])
                evict(qA[:, ch * B:(ch + 1) * B], pt2[:, :])

            # ================= P5: decode =================
            iota_i = st.tile([128, NPAIR * CHW], DT.int32)
            nc.gpsimd.iota(iota_i[:, :], pattern=[[1, NPAIR * CHW]], base=0,
                           channel_multiplier=0)
            iota_h = st.tile([128, NPAIR * CHW], F16)
            nc.vector.tensor_copy(iota_h[:, :], iota_i[:, :])

            girz_A = st.tile([128, 4 * B], BF16, name="girz_A")
            gin_A = st.tile([128, 2 * B], BF16, name="gin_A")
            # hoist uses wihT tiles [0:2]; ans q-part lives in tiles [2:4]
            for m in range(6):
                pp = ps.tile([128, B], F32, tag="bank", name=f"dxp{m}")
                for k in range(2):
                    nc.tensor.matmul(pp[:, :],
                                     W["ans_wihT"][2 + k][:, m * 128:(m + 1) * 128],
                                     qA[:, k * B:(k + 1) * B],
                                     start=(k == 0), stop=(k == 1))
                if m < 4:
                    bias = W["ans_brz"][:, m:m + 1]
                    dst = girz_A[:, m * B:(m + 1) * B]
                else:
                    bias = W["ans_bin"][:, m - 4:m - 3]
                    dst = gin_A[:, (m - 4) * B:(m - 3) * B]
                evict(dst, pp[:, :], bias)
            girz_Av = girz_A[:, :].rearrange("p (m t) -> p m t", m=4)
            gin_Av = gin_A[:, :].rearrange("p (m t) -> p m t", m=2)

            hid_ap = memA[:, :]
            last_ap = lastT0[:, :]
            out3 = out_logp.rearrange("(b t) v -> b t v", t=alen)
            W2K = NPAIR * CHW  # 2000 packed cols per partition-half

            Pd = gru2_hmm("ans", hid_ap, B, True, name="a0_")
            for t_step in range(alen):
                h2 = gru2_rest("ans", Pd, hid_ap, B, girz_Av, gin_Av,
                               rhs_x=last_ap, xk=(0, 2), name=f"a{t_step}_")
                hid_ap = h2[:, :]

                lg = wk.tile([128, W2K], BF16, tag="lg", bufs=1,
                             name=f"lg{t_step}")
                sext = wk.tile([128, NPAIR], F32, tag="sext", name=f"sx{t_step}")
                for j in range(NPAIR):
                    pl = ps.tile([128, CHW], F32, tag="bank", name=f"pl{t_step}_{j}")
                    for half in range(2):
                        oap = pl[half * B:(half + 1) * B, :]
                        for k in range(2):
                            nc.tensor.matmul(
                                oap, hid_ap[:, k * B:(k + 1) * B],
                                fcwT[k][:, (2 * j + half) * CHW:
                                         (2 * j + half + 1) * CHW],
                                start=(k == 0), stop=(k == 1),
                                tile_position=(0, 64) if half else None)
                    if fcb_nonzero:
                        nc.vector.tensor_add(pl[:, :], pl[:, :],
                                             fcbp[:, j * CHW:(j + 1) * CHW])
                    evict(lg[:, j * CHW:(j + 1) * CHW], pl[:, :])
                    dump = wk.tile([128, CHW], BF16, tag="dump", bufs=2,
                                   name=f"dump{t_step}_{j}")
                    nc.scalar.activation(dump[:, :], pl[:, :], AF.Exp,
                                         bias=zb[:, :], scale=1.0,
                                         accum_out=sext[:, j:j + 1])

                # full-width argmax over the packed [128, 4000] logit tile;
                # token = halfv[p] + col (fc weights host-permuted)
                c1 = wk.tile([128, 1], BF16, tag="c1", name=f"c1{t_step}")
                nc.vector.tensor_reduce(c1[:, :], lg[:, :],
                                        axis=mybir.AxisListType.X, op=ALU.max)
                msk = wk.tile([128, W2K], F16, tag="msk", bufs=1,
                              name=f"msk{t_step}")
                nc.vector.scalar_tensor_tensor(msk[:, :], lg[:, :], c1[:, :],
                                               iota_h[:, :], ALU.is_equal,
                                               ALU.mult)
                itv = wk.tile([128, 1], F32, tag="itv", name=f"itv{t_step}")
                nc.vector.tensor_reduce(itv[:, :], msk[:, :],
                                        axis=mybir.AxisListType.X, op=ALU.max)
                pack = wk.tile([128, 4], F32, tag="pack", name=f"pack{t_step}")
                nc.vector.tensor_copy(pack[:, 0:1], c1[:, :])
                nc.vector.tensor_add(pack[:, 1:2], itv[:, :], halfv[:, :])
                nc.vector.tensor_reduce(pack[:, 2:3], sext[:, :],
                                        axis=mybir.AxisListType.X, op=ALU.add)
                nc.vector.tensor_copy(pack[:, 3:4], zb[:, :])
                nc.sync.dma_start(cc_top_in[t_step][:, :], pack[:, :])
                nc.gpsimd.collective_compute("AllGather", ALU.bypass,
                                             ins=[cc_top_in[t_step][:, :]],
                                             outs=[cc_top_out[t_step][:, :]],
                                             replica_groups=rg)
                # next step's h-side matmuls run on the PE during the AG
                if t_step < alen - 1:
                    Pd = gru2_hmm("ans", hid_ap, B, True, name=f"a{t_step + 1}_")
                # readback: row = core*128 + half*64 + batch
                topall = wk.tile([B, 16 * 4], F32, tag="topall",
                                 name=f"topall{t_step}")
                nc.sync.dma_start(
                    topall[:, :].rearrange("b (c v) -> b c v", v=4),
                    cc_top_out[t_step].rearrange("(c h b) v -> b (c h) v",
                                                 h=2, b=B),
                )
                t3 = topall[:, :].rearrange("b (c v) -> b c v", v=4)
                gv = wk.tile([B, 1], F32, tag="gv", name=f"gv{t_step}")
                nc.vector.tensor_reduce(gv[:, :], t3[:, :, 0],
                                        axis=mybir.AxisListType.X, op=ALU.max)
                wtokf = wk.tile([B, 16], F32, tag="wtokf", name=f"wtokf{t_step}")
                nc.vector.scalar_tensor_tensor(wtokf[:, :], t3[:, :, 0], gv[:, :],
                                               t3[:, :, 1], ALU.is_equal, ALU.mult)
                wtok = wk.tile([B, 1], F32, tag="wtok", name=f"wtok{t_step}")
                nc.vector.tensor_reduce(wtok[:, :], wtokf[:, :],
                                        axis=mybir.AxisListType.X, op=ALU.max)
                stot = wk.tile([B, 1], F32, tag="stot", name=f"stot{t_step}")
                nc.vector.tensor_reduce(stot[:, :], t3[:, :, 2],
                                        axis=mybir.AxisListType.X, op=ALU.add)
                nlz = wk.tile([128, 1], F32, tag="nlz", name=f"nlz{t_step}")
                nc.scalar.activation(nlz[0:B, :], stot[:, :], AF.Ln,
                                     bias=zb[0:B, :], scale=1.0)
                nc.vector.tensor_scalar_mul(nlz[0:B, :], nlz[0:B, :], -1.0)
                nc.sync.dma_start(nlz[B:2 * B, :], nlz[0:B, :])
                for j in range(NPAIR):
                    og = wk.tile([128, CHW], F32, tag="og", bufs=2,
                                 name=f"og{t_step}_{j}")
                    if j % 2 == 0:
                        nc.scalar.activation(og[:, :], lg[:, j * CHW:(j + 1) * CHW],
                                             AF.Identity, bias=nlz[:, :], scale=1.0)
                    else:
                        nc.vector.tensor_scalar(og[:, :], lg[:, j * CHW:(j + 1) * CHW],
                                                nlz[:, :], None, ALU.add)
                    nc.sync.dma_start(out3[:, t_step, j * CHW:(j + 1) * CHW],
                                      og[0:B, :])
                    nc.sync.dma_start(
                        out3[:, t_step, 2000 + j * CHW:2000 + (j + 1) * CHW],
                        og[B:2 * B, :])
                if t_step == alen - 1:
                    continue
                wtoki = wk.tile([B, 1], DT.int32, tag="wtoki", name=f"wtoki{t_step}")
                nc.vector.tensor_copy(wtoki[:, :], wtok[:, :])
                lemb = wk.tile([B, E], BF16, tag="lemb", name=f"lemb{t_step}")
                nc.gpsimd.indirect_dma_start(
                    out=lemb[:, :], out_offset=None, in_=io["embed"][:, :],
                    in_offset=bass.IndirectOffsetOnAxis(ap=wtoki[:, :1], axis=0),
                )
                newlast = wk.tile([128, 2 * B], BF16, tag="lastT", bufs=2,
                                  name=f"lastT{t_step}")
                for ch in range(2):
                    pt = ps.tile([128, B], BF16, tag="bank", name=f"lt{t_step}_{ch}")
                    nc.tensor.transpose(pt[:, :], lemb[:, ch * 128:(ch + 1) * 128],
                                        ident[:B, :B])
                    evict(newlast[:, ch * B:(ch + 1) * B], pt[:, :])
                last_ap = newlast[:, :]

    nc.finalize()
    return nc


def prep_inputs(inputs):
    """Host-side shard/pack. Returns in_maps list for the 8 cores."""
    f32 = np.float32
    bf16 = ml_dtypes.bfloat16
    emb = np.ascontiguousarray(np.asarray(inputs["embed_w"], f32).astype(bf16))
    packs = {}
    for g in GK:
        wih = np.asarray(inputs[f"{g}_wih"], dtype=f32)
        whh = np.asarray(inputs[f"{g}_whh"], dtype=f32)
        bih = np.asarray(inputs[f"{g}_bih"], dtype=f32)
        bhh = np.asarray(inputs[f"{g}_bhh"], dtype=f32)
        brz = bih[:512] + bhh[:512]
        packs[f"{g}_wihT"] = np.ascontiguousarray(wih.T.astype(bf16))
        packs[f"{g}_whhT"] = np.ascontiguousarray(whh.T.astype(bf16))
        packs[f"{g}_brz"] = np.ascontiguousarray(brz.reshape(4, 128).T)
        packs[f"{g}_bin"] = np.ascontiguousarray(bih[512:768].reshape(2, 128).T)
        packs[f"{g}_bhn"] = np.ascontiguousarray(bhh[512:768].reshape(2, 128).T)
    packs["g1T"] = np.ascontiguousarray(np.asarray(inputs["gate_w1"], f32).T.astype(bf16))
    packs["g2T"] = np.ascontiguousarray(np.asarray(inputs["gate_w2"], f32).T.astype(bf16))
    packs["gb1"] = np.ascontiguousarray(np.asarray(inputs["gate_b1"], f32).reshape(2, 128).T)
    packs["gb2"] = np.asarray(inputs["gate_b2"], f32).reshape(1, 1)
    fcwT = np.ascontiguousarray(np.asarray(inputs["fc_w"], f32).T.astype(bf16))
    fcb = np.asarray(inputs["fc_b"], f32)
    fcb_nonzero = bool(np.any(fcb != 0))
    last0T = np.ascontiguousarray(
        np.tile(np.asarray(inputs["embed_w"], f32)[SEQBEGIN][:, None],
                (1, B)).astype(bf16))
    allfacts = np.asarray(inputs["allfacts"], np.int32)
    questions = np.asarray(inputs["questions"], np.int32)

    in_maps = []
    for j in range(NCORE):
        m = dict(packs)
        m["embed"] = emb
        # permute the vocab shard so packed col c on partition-half h maps to
        # token j*VS + h*2000 + c: new[:, jj*1000 + h*500 + r] = old h*2000+jj*500+r
        shard = fcwT[:, j * VS:(j + 1) * VS]
        sh3 = shard.reshape(E, 2, NPAIR, CHW)          # (h, jj, r)
        m["fcwT"] = np.ascontiguousarray(
            sh3.transpose(0, 2, 1, 3).reshape(E, VS))  # (jj, h, r)
        hv = np.zeros((128, 1), f32)
        hv[0:B, 0] = j * VS
        hv[B:128, 0] = j * VS + 2000
        m["halfv"] = hv
        if fcb_nonzero:
            fcbp = np.zeros((128, NPAIR * CHW), f32)
            sh = fcb[j * VS:(j + 1) * VS]
            for jj in range(NPAIR):
                fcbp[0:B, jj * CHW:(jj + 1) * CHW] = sh[jj * CHW:(jj + 1) * CHW]
                fcbp[B:128, jj * CHW:(jj + 1) * CHW] = \
                    sh[2000 + jj * CHW:2000 + (jj + 1) * CHW]
            m["fcbp"] = fcbp
        m["last0T"] = last0T
        m["facts_idx"] = np.ascontiguousarray(
            allfacts[j * BB:(j + 1) * BB].reshape(NSEQ, FL).T.reshape(-1, 1))
        m["q_idx"] = np.ascontiguousarray(
            questions[j * BB:(j + 1) * BB].reshape(BB, QL).T.reshape(-1, 1))
        in_maps.append(m)
    return in_maps, fcb_nonzero


_CACHE = {}


def kernel(**inputs):
    alen = int(inputs["alen"])
    in_maps, fcb_nonzero = prep_inputs(inputs)
    key = (alen, fcb_nonzero)
    if key not in _CACHE:
        _CACHE[key] = build_nc(alen, fcb_nonzero)
    nc = _CACHE[key]
    res = run_bass_kernel_spmd(nc, in_maps, core_ids=list(range(NCORE)))
    out = np.concatenate([res.results[j]["out_logp"] for j in range(NCORE)], axis=1)
    return out.astype(np.float32)
